# revision 94
# baseline (speedup 1.0000x reference)
import numpy as np
import ml_dtypes

import jax

# persistent XLA executable cache: the PJRT path retraces/relowers the
# wrapper every call, and without this it re-runs the NEFF packager too
jax.config.update("jax_compilation_cache_dir", "/tmp/jax_bass_cache")
jax.config.update("jax_persistent_cache_min_compile_time_secs", 0.0)
jax.config.update("jax_persistent_cache_min_entry_size_bytes", 0)

from concourse import bass, bacc, tile, mybir
from concourse.bass_utils import run_bass_kernel_spmd
from concourse.masks import make_identity

F32 = mybir.dt.float32
F32R = mybir.dt.float32r
BF16 = mybir.dt.bfloat16
ADD = mybir.AluOpType.add
SUB = mybir.AluOpType.subtract
MULT = mybir.AluOpType.mult
BYP = mybir.AluOpType.bypass
AF = mybir.ActivationFunctionType

B, S, H = 4, 2048, 512
BS = B * S                  # 8192 tokens
NCORE = 8
T = BS // NCORE             # 1024 tokens per core
HE = 2048
CC = 0.1 * 2.0 / (H * 8)    # MAX_LR * 2/(H*C): per-token grad scale
NT = T // 128               # 8 token blocks
NI = H // 128               # 4 feature blocks
NJ = HE // 128              # 16 hidden blocks
NCH = 4                     # backward chunks over HE
CW = HE // NCH              # 512
TH = T // 512               # 2 token halves (N=512 matmul limit)

# packed AllReduce buffer (bf16 elements): dW2T | dW1T | db1 | db2
OF_W2 = 0
OF_W1 = HE * H
OF_B1 = 2 * HE * H
OF_B2 = OF_B1 + HE
AR_N = OF_B2 + H

# gathered weight blob (bf16-element container, mixed content), 1/8 per core:
#   mw1 nat fp8 | mw2 nat fp8 | wqT | wkT | wvT | gw | gb | bq | bk | vb
#   | mb1 | mb2[0] | f32 smalls bitcast as bf16 pairs | pad
HEH = HE * H
OFW1 = 0                    # [2, HE, H] fp8 -> HEH bf16-elems
OFW2 = HEH                  # [2, H, HE] fp8 -> HEH bf16-elems
OFPQ = 2 * HEH              # [H, H] (wq.T) bf16
OFPK = OFPQ + H * H
OFPV = OFPK + H * H
OFGW = OFPV + H * H         # [H, 4] gate weights (lr, f, m, 0)
OFGB = OFGW + 4 * H         # [4] gate biases
OFBQ = OFGB + 4             # [H]
OFBK = OFBQ + H             # [H]
OFVB = OFBK + H             # [H]  (bv - mb2[1])
OFMB1 = OFVB + H            # [2, HE]
OFMB20 = OFMB1 + 2 * HE     # [H]  (mb2[0])
OFSML = OFMB20 + H          # f32 smalls as bf16 pairs (2*SMN bf16 elems)

# f32 smalls (element offsets within the f32 view): master biases in f32
OS_B1F = 0                      # [2, 128, NJ]
OS_B2F = OS_B1F + 2 * 128 * NJ  # [2, 128, NI]
OS_B2R = OS_B2F + 2 * 128 * NI  # [2, H]
SMN = OS_B2R + 2 * H            # 6144 f32

WTOT = ((OFSML + 2 * SMN + 1023) // 1024) * 1024
WSH = WTOT // NCORE         # per-core shard of the gathered blob
XN = T * H                  # 524288 int8 x elements per core
XSO = XN                    # x scales: T f32 (as 4*T i8 bytes)
WBO = XN + 4 * T            # weight shard bytes start (even offset)
IN_N = WBO + 2 * WSH        # single per-core int8 input tensor
HW2 = H + 8                 # y row: 512 int8 + 4B f32 scale + 4B pad

_CACHE = {}


def _build():
    nc = bacc.Bacc(num_devices=NCORE)

    I8 = mybir.dt.int8
    F8 = mybir.dt.float8e4
    inp = nc.declare_dram_parameter("inp", [IN_N], I8, isOutput=False)
    yout = nc.declare_dram_parameter("y", [T, HW2], I8, isOutput=True)
    xqv = inp[0:XN].rearrange("(t h) -> t h", h=H)
    xsv = inp[XSO:XSO + 4 * T].bitcast(F32).rearrange("(t a) -> t a", a=1)

    with tile.TileContext(nc, num_cores=NCORE, pool_alloc_mode="queue") as tc:
        # ---------- pools ----------
        pc = tc.alloc_tile_pool(name="consts", bufs=1)
        p_scr = tc.alloc_tile_pool(name="scr", bufs=2)
        pd = tc.alloc_tile_pool(name="dram", bufs=1, space="DRAM")
        pp_mm = tc.alloc_tile_pool(name="pmm", bufs=4, space="PSUM")
        pp_tr = tc.alloc_tile_pool(name="ptr", bufs=2, space="PSUM")
        pp_aux = tc.alloc_tile_pool(name="paux", bufs=1, space="PSUM")

        def psmm():
            return pp_mm.tile([128, 512], F32, name="pm", tag="mm")

        def pstr(dt=F32):
            return pp_tr.tile([128, 128], dt, name="pt", tag="tr")

        def psax(name):
            return pp_aux.tile([128, 512], F32, name=name, tag="aux")

        # ---------- consts ----------
        ident_f = pc.tile([128, 128], F32, name="ident_f")
        make_identity(nc, ident_f)
        ident_b = pc.tile([128, 128], BF16, name="ident_b")
        make_identity(nc, ident_b)
        ones_r_f = pc.tile([1, 128], F32, name="ones_r_f")
        nc.vector.memset(ones_r_f, 1.0)
        ones_r_b = pc.tile([1, 128], BF16, name="ones_r_b")
        nc.vector.memset(ones_r_b, 1.0)
        ones_c_f = pc.tile([128, 1], F32, name="ones_c_f")
        nc.vector.memset(ones_c_f, 1.0)
        ones_c_b = pc.tile([128, 1], BF16, name="ones_c_b")
        nc.vector.memset(ones_c_b, 1.0)

        m_t = [pc.tile([128, 1], F32, name=f"m_t{t}") for t in range(NT)]
        db21r = pc.tile([1, H], BF16, name="db21r")
        db20r = pc.tile([1, H], BF16, name="db20r")

        # ---------- dram scratch ----------
        ar0_in = pd.tile([1, 3], F32, name="ar0_in")
        ar0_out = pd.tile([1, 3], F32, name="ar0_out", addr_space="Shared")
        ar1_in = pd.tile([AR_N], BF16, name="ar1_in")
        ar1_out = pd.tile([AR_N], BF16, name="ar1_out", addr_space="Shared")
        ar2_in = pd.tile([AR_N], BF16, name="ar2_in")
        ar2_out = pd.tile([AR_N], BF16, name="ar2_out", addr_space="Shared")
        qf_d = pd.tile([H, T], F32R, name="qf_d")
        qt_d = pd.tile([T, H], F32, name="qt_d")
        wall = pd.tile([WTOT], BF16, name="wall", addr_space="Shared")
        w1tb_d = pd.tile([2, H, HE], BF16, name="w1tb_d")
        w2tb_d = pd.tile([2, HE, H], BF16, name="w2tb_d")
        w1n1_d = pd.tile([HE, H], BF16, name="w1n1_d")     # mw1[1] natural bf16
        w2nb_d = pd.tile([2, H, HE], BF16, name="w2nb_d")  # mw2 natural bf16

        def arview_w2(buf):
            return buf[OF_W2:OF_W2 + HE * H].rearrange("(a b) -> a b", b=H)

        def arview_w1(buf):
            return buf[OF_W1:OF_W1 + H * HE].rearrange("(a b) -> a b", b=HE)

        def arview_b1(buf):
            return buf[OF_B1:OF_B1 + HE].rearrange("(a b) -> a b", a=1)

        def arview_b2(buf):
            return buf[OF_B2:OF_B2 + H].rearrange("(a b) -> a b", a=1)

        wallq = wall.bitcast(F8)  # fp8 view of the blob (offsets in bytes)

        def wv_w1(d):  # mw1[d] natural [HE, H] fp8
            off = 2 * OFW1 + d * HE * H
            return wallq[off:off + HE * H].rearrange("(a b) -> a b", b=H)

        def wv_w2(d):  # mw2[d] natural [H, HE] fp8
            off = 2 * OFW2 + d * H * HE
            return wallq[off:off + H * HE].rearrange("(a b) -> a b", b=HE)

        wv_pq = wall[OFPQ:OFPQ + H * H].rearrange("(a b) -> a b", b=H)
        wv_pk = wall[OFPK:OFPK + H * H].rearrange("(a b) -> a b", b=H)
        wv_pv = wall[OFPV:OFPV + H * H].rearrange("(a b) -> a b", b=H)

        def mm_group(out, pairs, bias=None, fr=False):
            n = len(pairs)
            for i, (l, r) in enumerate(pairs):
                nc.tensor.matmul(out, l, r, start=(i == 0),
                                 stop=(i == n - 1 and bias is None))
            if bias is not None:
                l, r = bias
                nc.tensor.matmul(out, l, r, start=False, stop=True)

        # =======================================================
        # P0: AllGather sharded weights; build transposed stagings
        # =======================================================
        wsh_cp = pd.tile([WSH], BF16, name="wsh_cp")
        nc.sync.dma_start(wsh_cp, inp[WBO:WBO + 2 * WSH].bitcast(BF16))
        nc.gpsimd.collective_compute(
            "AllGather", BYP, replica_groups=[list(range(NCORE))],
            ins=[wsh_cp.opt()], outs=[wall.opt()])
        sml = wall[OFSML:OFSML + 2 * SMN].bitcast(F32)

        # bf16 smalls straight out of the gathered blob
        gw_s = pc.tile([128, 4 * NI], BF16, name="gw_s")
        wv_gw = wall[OFGW:OFGW + 4 * H].rearrange("(a b) -> a b", b=4)
        for it in range(NI):
            nc.sync.dma_start(gw_s[:, 4 * it:4 * it + 4],
                              wv_gw[it * 128:(it + 1) * 128, :])
        gb_s = pc.tile([1, 4], BF16, name="gb_s")
        nc.sync.dma_start(gb_s, wall[OFGB:OFGB + 4].rearrange("(a b) -> a b", a=1))
        b1rb_s = []
        for d in range(2):
            t3 = pc.tile([1, HE], BF16, name=f"b1rb_s{d}")
            nc.sync.dma_start(t3, wall[OFMB1 + d * HE:OFMB1 + (d + 1) * HE]
                              .rearrange("(a b) -> a b", a=1))
            b1rb_s.append(t3)
        b2rb_s = pc.tile([1, H], BF16, name="b2rb_s")
        nc.sync.dma_start(b2rb_s, wall[OFMB20:OFMB20 + H].rearrange("(a b) -> a b", a=1))
        b1f_s = []
        b2f_s = []
        b2r_s = []
        for d in range(2):
            t1 = pc.tile([128, NJ], F32, name=f"b1f_s{d}")
            nc.sync.dma_start(t1, sml[OS_B1F + d * 128 * NJ:OS_B1F + (d + 1) * 128 * NJ]
                              .rearrange("(p a) -> p a", a=NJ))
            b1f_s.append(t1)
            t2 = pc.tile([128, NI], F32, name=f"b2f_s{d}")
            nc.sync.dma_start(t2, sml[OS_B2F + d * 128 * NI:OS_B2F + (d + 1) * 128 * NI]
                              .rearrange("(p a) -> p a", a=NI))
            b2f_s.append(t2)
            t4 = pc.tile([1, H], F32, name=f"b2r_s{d}")
            nc.sync.dma_start(t4, sml[OS_B2R + d * H:OS_B2R + (d + 1) * H]
                              .rearrange("(a b) -> a b", a=1))
            b2r_s.append(t4)

        # long-lived P1 pools claimed first so the transients below stay
        # adjacent in the ring and merge into one reusable gap
        p_k = tc.alloc_tile_pool(name="pk", bufs=1)
        p_v = tc.alloc_tile_pool(name="pv", bufs=1, side="right")

        p_wst = tc.alloc_tile_pool(name="pwst", bufs=2, side="right")
        for d in range(2):
            # mw1[d] [HE, H] fp8 -> dequant -> w1tb_d[d] = mw1[d].T [H, HE] bf16
            for hi in range(NI):
                src = p_wst.tile([128, NJ, 128], F8, name=f"w1s{d}{hi}", tag="w1src")
                nc.sync.dma_start(src, wv_w1(d)[:, hi * 128:(hi + 1) * 128]
                                  .rearrange("(j p) c -> p j c", p=128))
                strip = p_wst.tile([128, HE], BF16, name=f"w1str{d}{hi}", tag="w1str")
                for j in range(NJ):
                    deq = p_wst.tile([128, 128], BF16, name=f"wdq{d}{hi}{j}", tag="wdq")
                    nc.scalar.activation(deq, src[:, j, :], AF.Copy)
                    pt = pstr(BF16)
                    nc.tensor.transpose(pt, deq, ident_b)
                    nc.scalar.activation(strip[:, j * 128:(j + 1) * 128], pt, AF.Copy)
                nc.sync.dma_start(w1tb_d[d][hi * 128:(hi + 1) * 128, :], strip)
        for d in range(2):
            # mw2[d] [H, HE] fp8 -> dequant -> w2tb_d[d] = mw2[d].T [HE, H] bf16
            for jt in range(NJ):
                src = p_wst.tile([128, NI, 128], F8, name=f"w2s{d}{jt}", tag="w2src")
                nc.sync.dma_start(src, wv_w2(d)[:, jt * 128:(jt + 1) * 128]
                                  .rearrange("(i p) c -> p i c", p=128))
                strip = p_wst.tile([128, H], BF16, name=f"w2str{d}{jt}", tag="w2str")
                for i in range(NI):
                    deq = p_wst.tile([128, 128], BF16, name=f"wdq2{d}{jt}{i}", tag="wdq")
                    nc.scalar.activation(deq, src[:, i, :], AF.Copy)
                    pt = pstr(BF16)
                    nc.tensor.transpose(pt, deq, ident_b)
                    nc.scalar.activation(strip[:, i * 128:(i + 1) * 128], pt, AF.Copy)
                nc.sync.dma_start(w2tb_d[d][jt * 128:(jt + 1) * 128, :], strip)
        # natural-layout bf16 copies for the backward passes
        for jt in range(NJ):
            t8 = p_wst.tile([128, H], F8, name=f"n1i{jt}", tag="n1i")
            nc.sync.dma_start(t8, wv_w1(1)[jt * 128:(jt + 1) * 128, :])
            tb = p_wst.tile([128, H], BF16, name=f"n1b{jt}", tag="n1b")
            nc.scalar.activation(tb, t8, AF.Copy)
            nc.sync.dma_start(w1n1_d[jt * 128:(jt + 1) * 128, :], tb)
        for d in range(2):
            for ot in range(NI):
                t8 = p_wst.tile([128, HE], F8, name=f"n2i{d}{ot}", tag="n2i")
                nc.sync.dma_start(t8, wv_w2(d)[ot * 128:(ot + 1) * 128, :])
                tb = p_wst.tile([128, HE], BF16, name=f"n2b{d}{ot}", tag="n2b")
                nc.scalar.activation(tb, t8, AF.Copy)
                nc.sync.dma_start(w2nb_d[d][ot * 128:(ot + 1) * 128, :], tb)
        p_wst.release()

        # =======================================================
        # P1: projections q/k/v + gates   (bf16 x, bf16 weights)
        # =======================================================
        k_fb = [p_k.tile([128, T], BF16, name=f"k_fb{i}") for i in range(NI)]
        k_tb = [p_k.tile([128, H], BF16, name=f"k_tb{t}") for t in range(NT)]

        p_x = tc.alloc_tile_pool(name="px", bufs=1)
        x_f = [p_x.tile([128, T], BF16, name=f"x_f{it}") for it in range(NI)]
        p_xt = tc.alloc_tile_pool(name="pxt", bufs=2, side="right")
        for tb in range(NT):
            xq = p_xt.tile([128, H], I8, name=f"xq{tb}", tag="xq")
            nc.sync.dma_start(xq, xqv[tb * 128:(tb + 1) * 128, :])
            xs = p_xt.tile([128, 1], F32, name=f"xs{tb}", tag="xs")
            nc.sync.dma_start(xs, xsv[tb * 128:(tb + 1) * 128, :])
            xt = p_xt.tile([128, H], BF16, name=f"xt{tb}", tag="xt")
            nc.scalar.activation(xt, xq, AF.Copy, scale=xs)
            for it in range(NI):
                pt = pstr(BF16)
                nc.tensor.transpose(pt, xt[:, it * 128:(it + 1) * 128], ident_b)
                nc.scalar.activation(x_f[it][:, tb * 128:(tb + 1) * 128], pt, AF.Copy)
        p_xt.release()

        p_wp = tc.alloc_tile_pool(name="pwp", bufs=1)
        wq_s = []
        wk_s = []
        wv_s = []
        for it in range(NI):
            t = p_wp.tile([128, H], BF16, name=f"wq_s{it}")
            nc.sync.dma_start(t, wv_pq[it * 128:(it + 1) * 128, :])
            wq_s.append(t)
            t = p_wp.tile([128, H], BF16, name=f"wk_s{it}")
            nc.sync.dma_start(t, wv_pk[it * 128:(it + 1) * 128, :])
            wk_s.append(t)
            t = p_wp.tile([128, H], BF16, name=f"wv_s{it}")
            nc.sync.dma_start(t, wv_pv[it * 128:(it + 1) * 128, :])
            wv_s.append(t)
        bq_s = p_wp.tile([1, H], BF16, name="bq_s")
        nc.sync.dma_start(bq_s, wall[OFBQ:OFBQ + H].rearrange("(a b) -> a b", a=1))
        bk_s = p_wp.tile([1, H], BF16, name="bk_s")
        nc.sync.dma_start(bk_s, wall[OFBK:OFBK + H].rearrange("(a b) -> a b", a=1))
        vb_s = p_wp.tile([1, H], BF16, name="vb_s")
        nc.sync.dma_start(vb_s, wall[OFVB:OFVB + H].rearrange("(a b) -> a b", a=1))

        v_t = [p_v.tile([128, H], F32, name=f"v_t{t}") for t in range(NT)]

        gsum_p = psax("gsum_p")

        for tb in range(NT):
            ts = slice(tb * 128, (tb + 1) * 128)
            # ---- gates ----
            pg = psmm()
            mm_group(pg[:, 0:4], [(x_f[it][:, ts], gw_s[:, 4 * it:4 * it + 4]) for it in range(NI)],
                     bias=(ones_r_b, gb_s))
            sig = p_scr.tile([128, 3], F32, name=f"sig{tb}", tag="sig")
            nc.scalar.activation(sig, pg[:, 0:3], AF.Sigmoid)
            nc.vector.tensor_scalar_mul(m_t[tb], sig[:, 0:1], CC)
            nc.tensor.matmul(gsum_p[0:1, 0:3], ones_c_f, sig,
                             start=(tb == 0), stop=(tb == NT - 1))

            # ---- q ----
            pq = psmm()
            mm_group(pq, [(x_f[it][:, ts], wq_s[it]) for it in range(NI)],
                     bias=(ones_r_b, bq_s))
            sqq = p_scr.tile([128, 1], F32, name="sqq", tag="sq1")
            scq = p_scr.tile([128, 512], F32, name="scq", tag="s512")
            nc.scalar.activation(scq, pq, AF.Square, accum_out=sqq)
            nrq = p_scr.tile([128, 1], F32, name="nrq", tag="nr1")
            nc.scalar.activation(nrq, sqq, AF.Sqrt)
            nc.vector.tensor_scalar_max(nrq, nrq, 1e-12)
            rnq = p_scr.tile([128, 1], F32, name="rnq", tag="rn1")
            nc.vector.reciprocal(rnq, nrq)
            qt_tile = p_scr.tile([128, 512], F32, name="qt_tile", tag="qt")
            nc.vector.tensor_scalar_mul(qt_tile, pq, rnq)
            nc.scalar.dma_start(qt_d[ts, :], qt_tile)
            for it in range(NI):
                ptq = pstr()
                nc.tensor.transpose(ptq, qt_tile[:, it * 128:(it + 1) * 128], ident_f)
                qfs = p_scr.tile([128, 128], F32R, name="qfs", tag="qfs")
                nc.scalar.activation(qfs, ptq, AF.Copy)
                nc.scalar.dma_start(qf_d[it * 128:(it + 1) * 128, ts], qfs)

            # ---- k ----
            pk = psmm()
            mm_group(pk, [(x_f[it][:, ts], wk_s[it]) for it in range(NI)],
                     bias=(ones_r_b, bk_s))
            sqk = p_scr.tile([128, 1], F32, name="sqk", tag="sq1")
            sck = p_scr.tile([128, 512], F32, name="sck", tag="s512")
            nc.scalar.activation(sck, pk, AF.Square, accum_out=sqk)
            nrk = p_scr.tile([128, 1], F32, name="nrk", tag="nr1")
            nc.scalar.activation(nrk, sqk, AF.Sqrt)
            nc.vector.tensor_scalar_max(nrk, nrk, 1e-12)
            rnk = p_scr.tile([128, 1], F32, name="rnk", tag="rn1")
            nc.vector.reciprocal(rnk, nrk)
            nc.vector.tensor_scalar_mul(k_tb[tb], pk, rnk)
            for it in range(NI):
                ptk = pstr(BF16)
                nc.tensor.transpose(ptk, k_tb[tb][:, it * 128:(it + 1) * 128], ident_b)
                nc.scalar.activation(k_fb[it][:, ts], ptk, AF.Copy)

            # ---- v ----
            pv = psmm()
            mm_group(pv, [(x_f[it][:, ts], wv_s[it]) for it in range(NI)],
                     bias=(ones_r_b, vb_s))
            nc.vector.tensor_copy(v_t[tb], pv)

        gsum_s = pc.tile([1, 3], F32, name="gsum_s")
        nc.scalar.activation(gsum_s, gsum_p[0:1, 0:3], AF.Copy)
        nc.gpsimd.dma_start(ar0_in, gsum_s)
        nc.gpsimd.collective_compute(
            "AllReduce", ADD, replica_groups=[list(range(NCORE))],
            ins=[ar0_in.opt()], outs=[ar0_out.opt()])

        p_wp.release()
        p_x.release()

        # =======================================================
        # P2: forward k-path layer 0 (bf16)
        # =======================================================
        p_w1tb0 = tc.alloc_tile_pool(name="pw1tb0", bufs=1)
        w1tb0 = []
        for it in range(NI):
            t = p_w1tb0.tile([128, HE], BF16, name=f"w1tb0{it}")
            (nc.sync if it % 2 == 0 else nc.gpsimd).dma_start(t, w1tb_d[0][it * 128:(it + 1) * 128, :])
            w1tb0.append(t)
        p_w1tb1 = tc.alloc_tile_pool(name="pw1tb1", bufs=1)
        w1tb1 = []
        for it in range(NI):
            t = p_w1tb1.tile([128, HE], BF16, name=f"w1tb1{it}")
            (nc.gpsimd if it % 2 == 0 else nc.sync).dma_start(t, w1tb_d[1][it * 128:(it + 1) * 128, :])
            w1tb1.append(t)
        p_x1 = tc.alloc_tile_pool(name="px1", bufs=1)
        x1f = [p_x1.tile([128, T], BF16, name=f"x1f{i}") for i in range(NI)]
        x1t = [p_x1.tile([128, H], BF16, name=f"x1t{t}") for t in range(NT)]
        p_w2tb1 = tc.alloc_tile_pool(name="pw2tb1", bufs=1)
        w2tb1 = []
        for jt in range(NJ):
            t = p_w2tb1.tile([128, H], BF16, name=f"w2tb1{jt}")
            (nc.sync if jt % 2 == 0 else nc.gpsimd).dma_start(t, w2tb_d[1][jt * 128:(jt + 1) * 128, :])
            w2tb1.append(t)
        p_w2tb0 = tc.alloc_tile_pool(name="pw2tb0", bufs=1)
        w2tb0 = []
        for jt in range(NJ):
            t = p_w2tb0.tile([128, H], BF16, name=f"w2tb0{jt}")
            (nc.gpsimd if jt % 2 == 0 else nc.sync).dma_start(t, w2tb_d[0][jt * 128:(jt + 1) * 128, :])
            w2tb0.append(t)

        p_h0 = tc.alloc_tile_pool(name="ph0", bufs=1)
        h0f = [p_h0.tile([128, T], BF16, name=f"h0f{j}") for j in range(NJ)]
        for jt in range(NJ):
            for th in range(TH):
                hs = slice(th * 512, (th + 1) * 512)
                ph = psmm()
                mm_group(ph, [(w1tb0[it][:, jt * 128:(jt + 1) * 128], k_fb[it][:, hs])
                              for it in range(NI)])
                nc.scalar.activation(h0f[jt][:, hs], ph, AF.Silu,
                                     bias=b1f_s[0][:, jt:jt + 1])

        for it in range(NI):
            for th in range(TH):
                hs = slice(th * 512, (th + 1) * 512)
                px = psmm()
                mm_group(px, [(w2tb0[jt][:, it * 128:(it + 1) * 128], h0f[jt][:, hs])
                              for jt in range(NJ)])
                nc.vector.scalar_tensor_tensor(x1f[it][:, hs], px, b2f_s[0][:, it:it + 1],
                                               k_fb[it][:, hs], ADD, ADD)
        for tb in range(NT):
            ts = slice(tb * 128, (tb + 1) * 128)
            px = psmm()
            mm_group(px, [(h0f[jt][:, ts], w2tb0[jt]) for jt in range(NJ)],
                     bias=(ones_r_b, b2rb_s))
            nc.vector.tensor_tensor(x1t[tb], px, k_tb[tb], ADD)

        p_h0.release()
        p_w2tb0.release()

        # =======================================================
        # P3: forward layer 1 + g2
        # =======================================================
        p_h1 = tc.alloc_tile_pool(name="ph1", bufs=1)
        h1f = [p_h1.tile([128, T], BF16, name=f"h1f{j}") for j in range(NJ)]
        for jt in range(NJ):
            for th in range(TH):
                hs = slice(th * 512, (th + 1) * 512)
                ph = psmm()
                mm_group(ph, [(w1tb1[it][:, jt * 128:(jt + 1) * 128], x1f[it][:, hs])
                              for it in range(NI)])
                nc.scalar.activation(h1f[jt][:, hs], ph, AF.Silu,
                                     bias=b1f_s[1][:, jt:jt + 1])

        p_g2 = tc.alloc_tile_pool(name="pg2", bufs=1, side="right")
        g2t = [p_g2.tile([128, H], BF16, name=f"g2t{t}") for t in range(NT)]
        g2f = [p_g2.tile([128, T], BF16, name=f"g2f{i}") for i in range(NI)]
        db21_p = psax("db21_p")
        for tb in range(NT):
            ts = slice(tb * 128, (tb + 1) * 128)
            px = psmm()
            mm_group(px, [(h1f[jt][:, ts], w2tb1[jt]) for jt in range(NJ)])
            sc1 = p_scr.tile([128, 512], F32, name="sc1", tag="s512")
            nc.vector.tensor_sub(sc1, px, v_t[tb])
            nc.vector.tensor_tensor(sc1, sc1, x1t[tb], ADD)
            nc.vector.tensor_scalar_mul(g2t[tb], sc1, m_t[tb])
            nc.tensor.matmul(db21_p[0:1, 0:512], ones_c_b, g2t[tb],
                             start=(tb == 0), stop=(tb == NT - 1))
            for ot in range(NI):
                ptg = pstr(BF16)
                nc.tensor.transpose(ptg, g2t[tb][:, ot * 128:(ot + 1) * 128], ident_b)
                nc.scalar.activation(g2f[ot][:, ts], ptg, AF.Copy)

        nc.scalar.activation(db21r, db21_p[0:1, 0:512], AF.Copy)
        nc.sync.dma_start(arview_b2(ar1_in), db21r)

        p_h1.release()
        p_w2tb1.release()

        # =======================================================
        # P4: backward layer 1 (4 chunks over HE)
        # =======================================================
        p_gx1 = tc.alloc_tile_pool(name="pgx1", bufs=1, side="right")
        gx1f = [p_gx1.tile([128, T], F32, name=f"gx1f{i}") for i in range(NI)]
        for it in range(NI):
            nc.scalar.activation(gx1f[it], g2f[it], AF.Copy)

        p_ch = tc.alloc_tile_pool(name="pch", bufs=1, side="right")
        h1c = [p_ch.tile([128, CW], BF16, name=f"h1c{t}") for t in range(NT)]
        gp1c = [p_ch.tile([128, CW], BF16, name=f"gp1c{t}") for t in range(NT)]
        gp1f = [p_ch.tile([128, T], BF16, name=f"gp1f{j}") for j in range(NCH)]

        p_nat1a = tc.alloc_tile_pool(name="pnat1a", bufs=1)
        w1n1b = []
        for jt in range(NJ):
            t = p_nat1a.tile([128, H], BF16, name=f"w1n1b{jt}")
            (nc.sync if jt % 2 == 0 else nc.gpsimd).dma_start(t, w1n1_d[jt * 128:(jt + 1) * 128, :])
            w1n1b.append(t)
        p_nat1b = tc.alloc_tile_pool(name="pnat1b", bufs=1)
        w2n1b = []
        for ot in range(NI):
            t = p_nat1b.tile([128, HE], BF16, name=f"w2n1b{ot}")
            (nc.gpsimd if ot % 2 == 0 else nc.sync).dma_start(t, w2nb_d[1][ot * 128:(ot + 1) * 128, :])
            w2n1b.append(t)

        for c in range(NCH):
            cs = slice(c * CW, (c + 1) * CW)
            for tb in range(NT):
                ts = slice(tb * 128, (tb + 1) * 128)
                p1 = psmm()
                mm_group(p1, [(x1f[it][:, ts], w1tb1[it][:, cs]) for it in range(NI)],
                         bias=(ones_r_b, b1rb_s[1][:, cs]))
                nc.scalar.activation(h1c[tb], p1, AF.Silu)
                nc.scalar.activation(gp1c[tb], p1, AF.Derivative_silu)
                p2 = psmm()
                mm_group(p2, [(g2f[ot][:, ts], w2n1b[ot][:, cs]) for ot in range(NI)])
                nc.vector.tensor_tensor(gp1c[tb], p2, gp1c[tb], MULT)

            # dW2T_1 rows of this chunk
            for js in range(4):
                pw = psmm()
                mm_group(pw, [(h1c[tb][:, js * 128:(js + 1) * 128], g2t[tb])
                              for tb in range(NT)])
                wst = p_scr.tile([128, 512], BF16, name="wst", tag="wst")
                nc.scalar.activation(wst, pw, AF.Copy)
                nc.sync.dma_start(
                    arview_w2(ar1_in)[(c * 4 + js) * 128:(c * 4 + js + 1) * 128, :], wst)
            # dW1T_1 columns of this chunk
            for ib in range(NI):
                pw = psmm()
                mm_group(pw, [(x1t[tb][:, ib * 128:(ib + 1) * 128], gp1c[tb])
                              for tb in range(NT)])
                wst = p_scr.tile([128, 512], BF16, name="wst2", tag="wst")
                nc.scalar.activation(wst, pw, AF.Copy)
                nc.sync.dma_start(
                    arview_w1(ar1_in)[ib * 128:(ib + 1) * 128, cs], wst)
            # db1_1 chunk
            pb = psax(f"db11_p{c}")
            mm_group(pb[0:1, 0:CW], [(ones_c_b, gp1c[tb]) for tb in range(NT)])
            dbr = p_scr.tile([1, CW], BF16, name=f"db11r{c}", tag="dbr")
            nc.scalar.activation(dbr, pb[0:1, 0:CW], AF.Copy)
            nc.sync.dma_start(arview_b1(ar1_in)[:, cs], dbr)
            # gpre1 transposed (F layout) for gx1 chain
            for tb in range(NT):
                ts = slice(tb * 128, (tb + 1) * 128)
                for js in range(4):
                    ptp = pstr(BF16)
                    nc.tensor.transpose(ptp, gp1c[tb][:, js * 128:(js + 1) * 128], ident_b)
                    nc.scalar.activation(gp1f[js][:, ts], ptp, AF.Copy)
            # gx1 += gpre1 @ W1n[1]
            for ib in range(NI):
                for th in range(TH):
                    hs = slice(th * 512, (th + 1) * 512)
                    pg = psmm()
                    mm_group(pg, [(w1n1b[c * 4 + js][:, ib * 128:(ib + 1) * 128],
                                   gp1f[js][:, hs]) for js in range(4)])
                    nc.vector.tensor_tensor(gx1f[ib][:, hs], gx1f[ib][:, hs], pg, ADD)

        nc.gpsimd.collective_compute(
            "AllReduce", ADD, replica_groups=[list(range(NCORE))],
            ins=[ar1_in.opt()], outs=[ar1_out.opt()])

        p_nat1b.release()
        p_nat1a.release()
        p_x1.release()
        p_w1tb1.release()

        # =======================================================
        # P5: backward layer 0
        # =======================================================
        p_w2n0b = tc.alloc_tile_pool(name="pw2n0b", bufs=1)
        w2n0b = []
        for ot in range(NI):
            t = p_w2n0b.tile([128, HE], BF16, name=f"w2n0b{ot}")
            (nc.sync if ot % 2 == 0 else nc.gpsimd).dma_start(t, w2nb_d[0][ot * 128:(ot + 1) * 128, :])
            w2n0b.append(t)

        p_gx1b = tc.alloc_tile_pool(name="pgx1b", bufs=1, side="right")
        gx1fb = [p_gx1b.tile([128, T], BF16, name=f"gx1fb{i}") for i in range(NI)]
        gx1t = [p_gx1b.tile([128, H], BF16, name=f"gx1t{t}") for t in range(NT)]
        for it in range(NI):
            nc.scalar.activation(gx1fb[it], gx1f[it], AF.Copy)
        for tb in range(NT):
            ts = slice(tb * 128, (tb + 1) * 128)
            for ib in range(NI):
                ptx = pstr()
                nc.tensor.transpose(ptx, gx1f[ib][:, ts], ident_f)
                nc.vector.tensor_copy(gx1t[tb][:, ib * 128:(ib + 1) * 128], ptx)

        db20_p = psax("db20_p")
        mm_group(db20_p[0:1, 0:512], [(ones_c_b, gx1t[tb]) for tb in range(NT)])
        nc.scalar.activation(db20r, db20_p[0:1, 0:512], AF.Copy)
        nc.sync.dma_start(arview_b2(ar2_in), db20r)

        h0c = [p_ch.tile([128, CW], BF16, name=f"h0c{t}", tag=f"h1c{t}") for t in range(NT)]
        gp0c = [p_ch.tile([128, CW], BF16, name=f"gp0c{t}", tag=f"gp1c{t}") for t in range(NT)]

        for c in range(NCH):
            cs = slice(c * CW, (c + 1) * CW)
            for tb in range(NT):
                ts = slice(tb * 128, (tb + 1) * 128)
                p1 = psmm()
                mm_group(p1, [(k_fb[it][:, ts], w1tb0[it][:, cs]) for it in range(NI)],
                         bias=(ones_r_b, b1rb_s[0][:, cs]))
                nc.scalar.activation(h0c[tb], p1, AF.Silu)
                nc.scalar.activation(gp0c[tb], p1, AF.Derivative_silu)
                p2 = psmm()
                mm_group(p2, [(gx1fb[ot][:, ts], w2n0b[ot][:, cs]) for ot in range(NI)])
                nc.vector.tensor_tensor(gp0c[tb], p2, gp0c[tb], MULT)
            for js in range(4):
                pw = psmm()
                mm_group(pw, [(h0c[tb][:, js * 128:(js + 1) * 128], gx1t[tb])
                              for tb in range(NT)])
                wst = p_scr.tile([128, 512], BF16, name="wst3", tag="wst")
                nc.scalar.activation(wst, pw, AF.Copy)
                nc.sync.dma_start(
                    arview_w2(ar2_in)[(c * 4 + js) * 128:(c * 4 + js + 1) * 128, :], wst)
            for ib in range(NI):
                pw = psmm()
                mm_group(pw, [(k_tb[tb][:, ib * 128:(ib + 1) * 128], gp0c[tb])
                              for tb in range(NT)])
                wst = p_scr.tile([128, 512], BF16, name="wst4", tag="wst")
                nc.scalar.activation(wst, pw, AF.Copy)
                nc.sync.dma_start(
                    arview_w1(ar2_in)[ib * 128:(ib + 1) * 128, cs], wst)
            pb = psax(f"db10_p{c}")
            mm_group(pb[0:1, 0:CW], [(ones_c_b, gp0c[tb]) for tb in range(NT)])
            dbr = p_scr.tile([1, CW], BF16, name=f"db10r{c}", tag="dbr")
            nc.scalar.activation(dbr, pb[0:1, 0:CW], AF.Copy)
            nc.sync.dma_start(arview_b1(ar2_in)[:, cs], dbr)

        nc.gpsimd.collective_compute(
            "AllReduce", ADD, replica_groups=[list(range(NCORE))],
            ins=[ar2_in.opt()], outs=[ar2_out.opt()])

        p_w2n0b.release()
        p_w1tb0.release()
        p_k.release()
        p_gx1b.release()
        p_ch.release()
        p_gx1.release()
        p_g2.release()
        p_v.release()

        # =======================================================
        # P6/P7: fused weight update + final forward on q
        # stage A: depth 0, stage B: depth 1
        # =======================================================
        gs = pc.tile([1, 3], F32, name="gs")
        nc.gpsimd.dma_start(gs, ar0_out)
        s_sc = pc.tile([1, 1], F32, name="s_sc")
        nc.vector.tensor_scalar(s_sc, gs[:, 1:2], -1.0 / BS, 1.0, MULT, ADD)
        tb_sc = pc.tile([1, 1], F32, name="tb_sc")
        nc.vector.tensor_scalar_mul(tb_sc, gs[:, 0:1], 0.1 / BS)
        pb1 = psax("pb1")
        nc.tensor.matmul(pb1[:, 0:1], ones_r_f, s_sc, start=True, stop=True)
        nc.tensor.matmul(pb1[:, 1:2], ones_r_f, tb_sc, start=True, stop=True)
        s_bc = pc.tile([128, 1], F32, name="s_bc")
        nc.scalar.activation(s_bc, pb1[:, 0:1], AF.Copy)
        tb_bc = pc.tile([128, 1], F32, name="tb_bc")
        nc.scalar.activation(tb_bc, pb1[:, 1:2], AF.Copy)

        # ---- stage A (depth 0; grads in ar2_out) ----
        p_x1q = tc.alloc_tile_pool(name="px1q", bufs=1)
        x1qf = [p_x1q.tile([128, T], F32R, name=f"x1qf{i}") for i in range(NI)]
        x1qt = [p_x1q.tile([128, H], F32, name=f"x1qt{t}") for t in range(NT)]

        p_w0 = tc.alloc_tile_pool(name="pw0", bufs=1)
        p_rot = tc.alloc_tile_pool(name="prot", bufs=2)
        w10 = []
        for it in range(NI):
            t = p_w0.tile([128, HE], F32R, name=f"w10_{it}")
            for cb in range(NCH):
                cs = slice(cb * CW, (cb + 1) * CW)
                rb = p_rot.tile([128, CW], BF16, name=f"r10_{it}_{cb}", tag="rot")
                (nc.sync if cb % 2 == 0 else nc.gpsimd).dma_start(rb, w1tb_d[0][it * 128:(it + 1) * 128, cs])
                nc.scalar.activation(t[:, cs], rb, AF.Copy)
            w10.append(t)
        w20 = []
        for jt in range(NJ):
            rb = p_rot.tile([128, H], BF16, name=f"r20_{jt}", tag="rot")
            (nc.gpsimd if jt % 2 == 0 else nc.sync).dma_start(rb, w2tb_d[0][jt * 128:(jt + 1) * 128, :])
            t = p_w0.tile([128, H], F32R, name=f"w20_{jt}")
            nc.scalar.activation(t, rb, AF.Copy)
            w20.append(t)

        def update_weights(w1x, w2x, arw, d, pu):
            for it in range(NI):
                for cb in range(NCH):
                    cs = slice(cb * CW, (cb + 1) * CW)
                    g1 = pu.tile([128, CW], BF16, name=f"g1_{d}_{it}_{cb}", tag="g1")
                    nc.sync.dma_start(g1, arview_w1(arw)[it * 128:(it + 1) * 128, cs])
                    t1 = pu.tile([128, CW], F32, name=f"t1_{d}_{it}_{cb}", tag="t1")
                    nc.scalar.activation(t1, g1, AF.Copy, scale=tb_bc)
                    nc.vector.scalar_tensor_tensor(w1x[it][:, cs], w1x[it][:, cs],
                                                   s_bc, t1, MULT, SUB)
            for jt in range(NJ):
                g2_ = pu.tile([128, H], BF16, name=f"g2_{d}_{jt}", tag="g2")
                nc.sync.dma_start(g2_, arview_w2(arw)[jt * 128:(jt + 1) * 128, :])
                t2 = pu.tile([128, H], F32, name=f"t2_{d}_{jt}", tag="t2")
                nc.scalar.activation(t2, g2_, AF.Copy, scale=tb_bc)
                nc.vector.scalar_tensor_tensor(w2x[jt], w2x[jt], s_bc, t2, MULT, SUB)
            gb1 = pu.tile([128, NJ], BF16, name=f"gb1_{d}", tag="gb1")
            nc.sync.dma_start(gb1, arw[OF_B1:OF_B1 + HE].rearrange("(a p) -> p a", p=128))
            tb1 = pu.tile([128, NJ], F32, name=f"tb1_{d}", tag="tb1")
            nc.scalar.activation(tb1, gb1, AF.Copy, scale=tb_bc)
            nc.vector.scalar_tensor_tensor(b1f_s[d], b1f_s[d], s_bc, tb1, MULT, SUB)
            gb2 = pu.tile([128, NI], BF16, name=f"gb2_{d}", tag="gb2")
            nc.sync.dma_start(gb2, arw[OF_B2:OF_B2 + H].rearrange("(a p) -> p a", p=128))
            tb2 = pu.tile([128, NI], F32, name=f"tb2_{d}", tag="tb2")
            nc.scalar.activation(tb2, gb2, AF.Copy, scale=tb_bc)
            nc.vector.scalar_tensor_tensor(b2f_s[d], b2f_s[d], s_bc, tb2, MULT, SUB)
            gb2r = pu.tile([1, H], BF16, name=f"gb2r_{d}", tag="gb2r")
            nc.sync.dma_start(gb2r, arview_b2(arw))
            tb2r = pu.tile([1, H], F32, name=f"tb2r_{d}", tag="tb2r")
            nc.scalar.activation(tb2r, gb2r, AF.Copy, scale=tb_sc)
            nc.vector.scalar_tensor_tensor(b2r_s[d], b2r_s[d], s_sc, tb2r, MULT, SUB)

        p_updA = tc.alloc_tile_pool(name="pupdA", bufs=1)
        update_weights(w10, w20, ar2_out, 0, p_updA)

        p_q = tc.alloc_tile_pool(name="pq", bufs=1)
        qfh = []
        for it in range(NI):
            t = p_q.tile([128, T], F32R, name=f"qfh{it}")
            (nc.scalar if it % 2 == 0 else nc.gpsimd).dma_start(t, qf_d[it * 128:(it + 1) * 128, :])
            qfh.append(t)

        p_hq = tc.alloc_tile_pool(name="phq", bufs=1)
        p_hq2 = tc.alloc_tile_pool(name="phq2", bufs=1)
        for hb in range(TH):
            hs = slice(hb * 512, (hb + 1) * 512)
            h0q = []
            for jt in range(NJ):
                ph = psmm()
                mm_group(ph, [(w10[it][:, jt * 128:(jt + 1) * 128], qfh[it][:, hs])
                              for it in range(NI)])
                hqt = (p_hq if jt < 8 else p_hq2).tile(
                    [128, 512], F32R, name=f"h0q{jt}_{hb}", tag=f"h0q{jt}")
                nc.scalar.activation(hqt, ph, AF.Silu, bias=b1f_s[0][:, jt:jt + 1])
                h0q.append(hqt)
            for it in range(NI):
                px = psmm()
                mm_group(px, [(w20[jt][:, it * 128:(it + 1) * 128], h0q[jt])
                              for jt in range(NJ)])
                nc.vector.scalar_tensor_tensor(x1qf[it][:, hs], px, b2f_s[0][:, it:it + 1],
                                               qfh[it][:, hs], ADD, ADD)
            for tb4 in range(4):
                tbg = hb * 4 + tb4
                px = psmm()
                mm_group(px, [(h0q[jt][:, tb4 * 128:(tb4 + 1) * 128], w20[jt])
                              for jt in range(NJ)],
                         bias=(ones_r_f, b2r_s[0]))
                qtt = p_scr.tile([128, 512], F32, name=f"qtt{tbg}", tag="s512")
                nc.sync.dma_start(qtt, qt_d[tbg * 128:(tbg + 1) * 128, :])
                nc.vector.tensor_tensor(x1qt[tbg], px, qtt, ADD)

        p_hq2.release()
        p_hq.release()
        p_q.release()
        p_updA.release()
        p_rot.release()
        p_w0.release()

        # ---- stage B (depth 1; grads in ar1_out) ----
        p_w1x = tc.alloc_tile_pool(name="pw1x", bufs=1)
        p_rotb = tc.alloc_tile_pool(name="protb", bufs=2)
        w11 = []
        for it in range(NI):
            t = p_w1x.tile([128, HE], F32R, name=f"w11_{it}")
            for cb in range(NCH):
                cs = slice(cb * CW, (cb + 1) * CW)
                rb = p_rotb.tile([128, CW], BF16, name=f"r11_{it}_{cb}", tag="rot")
                (nc.sync if cb % 2 == 0 else nc.gpsimd).dma_start(rb, w1tb_d[1][it * 128:(it + 1) * 128, cs])
                nc.scalar.activation(t[:, cs], rb, AF.Copy)
            w11.append(t)
        w21 = []
        for jt in range(NJ):
            rb = p_rotb.tile([128, H], BF16, name=f"r21_{jt}", tag="rot")
            (nc.gpsimd if jt % 2 == 0 else nc.sync).dma_start(rb, w2tb_d[1][jt * 128:(jt + 1) * 128, :])
            t = p_w1x.tile([128, H], F32R, name=f"w21_{jt}")
            nc.scalar.activation(t, rb, AF.Copy)
            w21.append(t)

        p_updB = tc.alloc_tile_pool(name="pupdB", bufs=1)
        update_weights(w11, w21, ar1_out, 1, p_updB)

        p_h1q = tc.alloc_tile_pool(name="ph1q", bufs=1)
        p_h1q2 = tc.alloc_tile_pool(name="ph1q2", bufs=1)
        for hb in range(TH):
            hs = slice(hb * 512, (hb + 1) * 512)
            h1q = []
            for jt in range(NJ):
                ph = psmm()
                mm_group(ph, [(w11[it][:, jt * 128:(jt + 1) * 128], x1qf[it][:, hs])
                              for it in range(NI)])
                hqt = (p_h1q if jt < 8 else p_h1q2).tile(
                    [128, 512], F32R, name=f"h1q{jt}_{hb}", tag=f"h1q{jt}")
                nc.scalar.activation(hqt, ph, AF.Silu, bias=b1f_s[1][:, jt:jt + 1])
                h1q.append(hqt)
            for tb4 in range(4):
                tbg = hb * 4 + tb4
                py = psmm()
                mm_group(py, [(h1q[jt][:, tb4 * 128:(tb4 + 1) * 128], w21[jt])
                              for jt in range(NJ)],
                         bias=(ones_r_f, b2r_s[1]))
                nc.vector.tensor_tensor(x1qt[tbg], x1qt[tbg], py, ADD)
                # int8 per-token output: y = round(x * 127/rowmax), scale packed
                rmx = p_scr.tile([128, 1], F32, name=f"rmx{tbg}", tag="sq1")
                nc.vector.reduce_max(rmx, x1qt[tbg], axis=mybir.AxisListType.X,
                                     apply_absolute_value=True)
                nc.vector.tensor_scalar_max(rmx, rmx, 1e-20)
                ysc = p_scr.tile([128, 1], F32, name=f"ysc{tbg}", tag="nr1")
                nc.vector.tensor_scalar_mul(ysc, rmx, 1.0 / 127.0)
                yiv = p_scr.tile([128, 1], F32, name=f"yiv{tbg}", tag="rn1")
                nc.vector.reciprocal(yiv, ysc)
                yb = p_h1q2.tile([128, HW2], I8, name=f"yb{tbg}", tag="ybq")
                nc.scalar.activation(yb[:, 0:H], x1qt[tbg], AF.Copy, scale=yiv)
                nc.vector.tensor_copy(yb[:, H:H + 4].bitcast(F32), ysc)
                nc.vector.memset(yb[:, H + 4:HW2], 0)
                nc.sync.dma_start(yout[tbg * 128:(tbg + 1) * 128, :], yb)

        p_h1q2.release()
        p_h1q.release()
        p_updB.release()
        p_rotb.release()
        p_w1x.release()
        p_x1q.release()
        p_scr.release()
        pc.release()
        pp_aux.release()
        pp_tr.release()
        pp_mm.release()

    nc.finalize()
    return nc


def _get_nc():
    if "nc" not in _CACHE:
        nc = _build()
        # the module is immutable after finalize(); memoize its serialization
        # (recomputed inside every PJRT lowering otherwise, ~60ms/call)
        try:
            jb = nc.to_json_bytes()
            nc.to_json_bytes = lambda: jb
        except Exception:
            pass
        _CACHE["nc"] = nc
    return _CACHE["nc"]


_F8TBL = None


def _f8_pack(a_f32, out_i8):
    # f32 -> bf16 -> fp8 via lookup table (ml_dtypes' direct f32->fp8 cast
    # is slow); double rounding shifts values by at most one fp8 ulp
    global _F8TBL
    if _F8TBL is None:
        bits = np.arange(65536, dtype=np.uint16)
        with np.errstate(invalid="ignore"):
            _F8TBL = (bits.view(ml_dtypes.bfloat16)
                      .astype(ml_dtypes.float8_e4m3fn).view(np.uint8))
    b = np.asarray(a_f32, dtype=ml_dtypes.bfloat16).view(np.uint16).ravel()
    np.take(_F8TBL, b, out=out_i8.view(np.uint8))


def _prep(inputs):
    f32 = np.float32
    bf = ml_dtypes.bfloat16

    def g(n):
        return np.asarray(inputs[n], dtype=f32)

    x = g("x").reshape(BS, H)
    wq, bq = g("wq"), g("bq")
    wk, bk = g("wk"), g("bk")
    wv, bv = g("wv"), g("bv")
    wlr, blr = g("wlr"), g("blr")
    wf, bfg = g("wf"), g("bf")
    wm, bm = g("wm"), g("bm")
    mw1, mb1 = g("mw1"), g("mb1")
    mw2, mb2 = g("mw2"), g("mb2")

    bufs = _CACHE.setdefault("bufs", {})
    if not bufs:
        bufs["blob"] = np.zeros(WTOT, dtype=bf)
        bufs["xbuf"] = np.empty((BS, H), f32)
        bufs["xq"] = np.empty((BS, H), np.int8)
        bufs["ics"] = [np.empty(IN_N, np.int8) for _ in range(NCORE)]
    blob = bufs["blob"]
    blob8 = blob.view(np.int8)
    _f8_pack(mw1, blob8[2 * OFW1:2 * OFW1 + 2 * HEH])
    _f8_pack(mw2, blob8[2 * OFW2:2 * OFW2 + 2 * HEH])
    blob[OFPQ:OFPQ + H * H] = np.ascontiguousarray(wq.T).astype(bf).ravel()
    blob[OFPK:OFPK + H * H] = np.ascontiguousarray(wk.T).astype(bf).ravel()
    blob[OFPV:OFPV + H * H] = np.ascontiguousarray(wv.T).astype(bf).ravel()
    gwmat = np.concatenate([wlr, wf, wm, np.zeros((1, H), f32)], axis=0).T  # [H,4]
    blob[OFGW:OFGW + 4 * H] = np.ascontiguousarray(gwmat).astype(bf).ravel()
    blob[OFGB:OFGB + 4] = np.array([blr[0], bfg[0], bm[0], 0.0], f32).astype(bf)
    blob[OFBQ:OFBQ + H] = bq.astype(bf)
    blob[OFBK:OFBK + H] = bk.astype(bf)
    blob[OFVB:OFVB + H] = (bv - mb2[1]).astype(bf)
    blob[OFMB1:OFMB1 + 2 * HE] = mb1.astype(bf).ravel()
    blob[OFMB20:OFMB20 + H] = mb2[0].astype(bf)

    sm = np.zeros(SMN, dtype=f32)
    sm[OS_B1F:OS_B1F + 2 * 128 * NJ] = \
        mb1.reshape(2, NJ, 128).transpose(0, 2, 1).ravel()
    sm[OS_B2F:OS_B2F + 2 * 128 * NI] = \
        mb2.reshape(2, NI, 128).transpose(0, 2, 1).ravel()
    sm[OS_B2R:OS_B2R + 2 * H] = mb2.ravel()
    blob[OFSML:OFSML + 2 * SMN] = sm.view(bf)

    # int8 per-token x (in-place passes through persistent buffers)
    mx = np.maximum(x.max(axis=1), -x.min(axis=1))
    xsc = np.where(mx == 0, 1.0, mx * (1.0 / 127.0)).astype(f32)
    xb = bufs["xbuf"]
    np.multiply(x, (1.0 / xsc)[:, None], out=xb)
    # |x/s| <= 127*(1+2^-22), so rint stays within int8 range without a clip
    np.rint(xb, out=xb)
    xq_all = bufs["xq"]
    np.copyto(xq_all, xb, casting="unsafe")

    in_maps = []
    for cid in range(NCORE):
        ic = bufs["ics"][cid]
        ic[0:XN] = xq_all[cid * T:(cid + 1) * T].ravel()
        ic[XSO:XSO + 4 * T] = xsc[cid * T:(cid + 1) * T].view(np.int8)
        ic[WBO:WBO + 2 * WSH] = blob8[cid * 2 * WSH:(cid + 1) * 2 * WSH]
        in_maps.append({"inp": ic})
    return in_maps


def kernel(**inputs):
    nc = _get_nc()
    in_maps = _prep(inputs)
    res = run_bass_kernel_spmd(nc, in_maps, list(range(NCORE)))
    out = np.empty((BS, H), np.float32)
    for cid in range(NCORE):
        yr = np.asarray(res.results[cid]["y"])                      # [T, H+8] int8
        sc = np.ascontiguousarray(yr[:, H:H + 4]).view(np.float32)  # [T, 1]
        np.multiply(yr[:, 0:H], sc, out=out[cid * T:(cid + 1) * T])
    return out.reshape(B, S, H)


# revision 100
# speedup vs baseline: 1.0155x; 1.0155x over previous
import numpy as np
import ml_dtypes

import jax

# persistent XLA executable cache: the PJRT path retraces/relowers the
# wrapper every call, and without this it re-runs the NEFF packager too
jax.config.update("jax_compilation_cache_dir", "/tmp/jax_bass_cache")
jax.config.update("jax_persistent_cache_min_compile_time_secs", 0.0)
jax.config.update("jax_persistent_cache_min_entry_size_bytes", 0)

from concourse import bass, bacc, tile, mybir
from concourse.bass_utils import run_bass_kernel_spmd
from concourse.masks import make_identity

F32 = mybir.dt.float32
F32R = mybir.dt.float32r
BF16 = mybir.dt.bfloat16
ADD = mybir.AluOpType.add
SUB = mybir.AluOpType.subtract
MULT = mybir.AluOpType.mult
BYP = mybir.AluOpType.bypass
AF = mybir.ActivationFunctionType

B, S, H = 4, 2048, 512
BS = B * S                  # 8192 tokens
NCORE = 8
T = BS // NCORE             # 1024 tokens per core
HE = 2048
CC = 0.1 * 2.0 / (H * 8)    # MAX_LR * 2/(H*C): per-token grad scale
NT = T // 128               # 8 token blocks
NI = H // 128               # 4 feature blocks
NJ = HE // 128              # 16 hidden blocks
NCH = 4                     # backward chunks over HE
CW = HE // NCH              # 512
TH = T // 512               # 2 token halves (N=512 matmul limit)

# packed AllReduce buffer (bf16 elements): dW2T | dW1T | db1 | db2
OF_W2 = 0
OF_W1 = HE * H
OF_B1 = 2 * HE * H
OF_B2 = OF_B1 + HE
AR_N = OF_B2 + H

# gathered weight blob (bf16-element container, mixed content), 1/8 per core:
#   mw1 nat fp8 | mw2 nat fp8 | wqT | wkT | wvT | gw | gb | bq | bk | vb
#   | mb1 | mb2[0] | f32 smalls bitcast as bf16 pairs | pad
HEH = HE * H
OFW1 = 0                    # [2, HE, H] fp8 -> HEH bf16-elems
OFW2 = HEH                  # [2, H, HE] fp8 -> HEH bf16-elems
OFPQ = 2 * HEH              # [H, H] (wq.T) bf16
OFPK = OFPQ + H * H
OFPV = OFPK + H * H
OFGW = OFPV + H * H         # [H, 4] gate weights (lr, f, m, 0)
OFGB = OFGW + 4 * H         # [4] gate biases
OFBQ = OFGB + 4             # [H]
OFBK = OFBQ + H             # [H]
OFVB = OFBK + H             # [H]  (bv - mb2[1])
OFMB1 = OFVB + H            # [2, HE]
OFMB20 = OFMB1 + 2 * HE     # [H]  (mb2[0])
OFSML = OFMB20 + H          # f32 smalls as bf16 pairs (2*SMN bf16 elems)

# f32 smalls (element offsets within the f32 view): master biases in f32
OS_B1F = 0                      # [2, 128, NJ]
OS_B2F = OS_B1F + 2 * 128 * NJ  # [2, 128, NI]
OS_B2R = OS_B2F + 2 * 128 * NI  # [2, H]
SMN = OS_B2R + 2 * H            # 6144 f32

WTOT = ((OFSML + 2 * SMN + 1023) // 1024) * 1024
WSH = WTOT // NCORE         # per-core shard of the gathered blob
XN = T * H                  # 524288 int8 x elements per core
XSO = XN                    # x scales: T f32 (as 4*T i8 bytes)
WBO = XN + 4 * T            # weight shard bytes start (even offset)
IN_N = WBO + 2 * WSH        # single per-core int8 input tensor
HW2 = H + 8                 # y row: 512 int8 + 4B f32 scale + 4B pad

_CACHE = {}


def _build():
    nc = bacc.Bacc(num_devices=NCORE)

    I8 = mybir.dt.int8
    F8 = mybir.dt.float8e4
    inp = nc.declare_dram_parameter("inp", [IN_N], I8, isOutput=False)
    yout = nc.declare_dram_parameter("y", [T, HW2], I8, isOutput=True)
    xqv = inp[0:XN].rearrange("(t h) -> t h", h=H)
    xsv = inp[XSO:XSO + 4 * T].bitcast(F32).rearrange("(t a) -> t a", a=1)

    with tile.TileContext(nc, num_cores=NCORE, pool_alloc_mode="queue") as tc:
        # ---------- pools ----------
        pc = tc.alloc_tile_pool(name="consts", bufs=1)
        p_scr = tc.alloc_tile_pool(name="scr", bufs=2)
        pd = tc.alloc_tile_pool(name="dram", bufs=1, space="DRAM")
        pp_mm = tc.alloc_tile_pool(name="pmm", bufs=4, space="PSUM")
        pp_tr = tc.alloc_tile_pool(name="ptr", bufs=2, space="PSUM")
        pp_aux = tc.alloc_tile_pool(name="paux", bufs=1, space="PSUM")

        def psmm():
            return pp_mm.tile([128, 512], F32, name="pm", tag="mm")

        def pstr(dt=F32):
            return pp_tr.tile([128, 128], dt, name="pt", tag="tr")

        def psax(name):
            return pp_aux.tile([128, 512], F32, name=name, tag="aux")

        # ---------- consts ----------
        ident_f = pc.tile([128, 128], F32, name="ident_f")
        make_identity(nc, ident_f)
        ident_b = pc.tile([128, 128], BF16, name="ident_b")
        make_identity(nc, ident_b)
        ones_r_f = pc.tile([1, 128], F32, name="ones_r_f")
        nc.vector.memset(ones_r_f, 1.0)
        ones_r_b = pc.tile([1, 128], BF16, name="ones_r_b")
        nc.vector.memset(ones_r_b, 1.0)
        ones_c_f = pc.tile([128, 1], F32, name="ones_c_f")
        nc.vector.memset(ones_c_f, 1.0)
        ones_c_b = pc.tile([128, 1], BF16, name="ones_c_b")
        nc.vector.memset(ones_c_b, 1.0)

        m_t = [pc.tile([128, 1], F32, name=f"m_t{t}") for t in range(NT)]
        db21r = pc.tile([1, H], BF16, name="db21r")
        db20r = pc.tile([1, H], BF16, name="db20r")

        # ---------- dram scratch ----------
        ar0_in = pd.tile([1, 3], F32, name="ar0_in")
        ar0_out = pd.tile([1, 3], F32, name="ar0_out", addr_space="Shared")
        ar1_in = pd.tile([AR_N], BF16, name="ar1_in")
        ar1_out = pd.tile([AR_N], BF16, name="ar1_out", addr_space="Shared")
        ar2_in = pd.tile([AR_N], BF16, name="ar2_in")
        ar2_out = pd.tile([AR_N], BF16, name="ar2_out", addr_space="Shared")
        qf_d = pd.tile([H, T], F32R, name="qf_d")
        qt_d = pd.tile([T, H], F32, name="qt_d")
        wall = pd.tile([WTOT], BF16, name="wall", addr_space="Shared")
        w1tb_d = pd.tile([2, H, HE], BF16, name="w1tb_d")
        w2tb_d = pd.tile([2, HE, H], BF16, name="w2tb_d")
        w1n1_d = pd.tile([HE, H], BF16, name="w1n1_d")     # mw1[1] natural bf16
        w2nb_d = pd.tile([2, H, HE], BF16, name="w2nb_d")  # mw2 natural bf16

        def arview_w2(buf):
            return buf[OF_W2:OF_W2 + HE * H].rearrange("(a b) -> a b", b=H)

        def arview_w1(buf):
            return buf[OF_W1:OF_W1 + H * HE].rearrange("(a b) -> a b", b=HE)

        def arview_b1(buf):
            return buf[OF_B1:OF_B1 + HE].rearrange("(a b) -> a b", a=1)

        def arview_b2(buf):
            return buf[OF_B2:OF_B2 + H].rearrange("(a b) -> a b", a=1)

        wallq = wall.bitcast(F8)  # fp8 view of the blob (offsets in bytes)

        def wv_w1(d):  # mw1[d] natural [HE, H] fp8
            off = 2 * OFW1 + d * HE * H
            return wallq[off:off + HE * H].rearrange("(a b) -> a b", b=H)

        def wv_w2(d):  # mw2[d] natural [H, HE] fp8
            off = 2 * OFW2 + d * H * HE
            return wallq[off:off + H * HE].rearrange("(a b) -> a b", b=HE)

        wv_pq = wall[OFPQ:OFPQ + H * H].rearrange("(a b) -> a b", b=H)
        wv_pk = wall[OFPK:OFPK + H * H].rearrange("(a b) -> a b", b=H)
        wv_pv = wall[OFPV:OFPV + H * H].rearrange("(a b) -> a b", b=H)

        def mm_group(out, pairs, bias=None, fr=False):
            n = len(pairs)
            for i, (l, r) in enumerate(pairs):
                nc.tensor.matmul(out, l, r, start=(i == 0),
                                 stop=(i == n - 1 and bias is None))
            if bias is not None:
                l, r = bias
                nc.tensor.matmul(out, l, r, start=False, stop=True)

        # =======================================================
        # P0: AllGather sharded weights; build transposed stagings
        # =======================================================
        wsh_cp = pd.tile([WSH], BF16, name="wsh_cp")
        nc.sync.dma_start(wsh_cp, inp[WBO:WBO + 2 * WSH].bitcast(BF16))
        nc.gpsimd.collective_compute(
            "AllGather", BYP, replica_groups=[list(range(NCORE))],
            ins=[wsh_cp.opt()], outs=[wall.opt()])
        sml = wall[OFSML:OFSML + 2 * SMN].bitcast(F32)

        # bf16 smalls straight out of the gathered blob
        gw_s = pc.tile([128, 4 * NI], BF16, name="gw_s")
        wv_gw = wall[OFGW:OFGW + 4 * H].rearrange("(a b) -> a b", b=4)
        for it in range(NI):
            nc.sync.dma_start(gw_s[:, 4 * it:4 * it + 4],
                              wv_gw[it * 128:(it + 1) * 128, :])
        gb_s = pc.tile([1, 4], BF16, name="gb_s")
        nc.sync.dma_start(gb_s, wall[OFGB:OFGB + 4].rearrange("(a b) -> a b", a=1))
        b1rb_s = []
        for d in range(2):
            t3 = pc.tile([1, HE], BF16, name=f"b1rb_s{d}")
            nc.sync.dma_start(t3, wall[OFMB1 + d * HE:OFMB1 + (d + 1) * HE]
                              .rearrange("(a b) -> a b", a=1))
            b1rb_s.append(t3)
        b2rb_s = pc.tile([1, H], BF16, name="b2rb_s")
        nc.sync.dma_start(b2rb_s, wall[OFMB20:OFMB20 + H].rearrange("(a b) -> a b", a=1))
        b1f_s = []
        b2f_s = []
        b2r_s = []
        for d in range(2):
            t1 = pc.tile([128, NJ], F32, name=f"b1f_s{d}")
            nc.sync.dma_start(t1, sml[OS_B1F + d * 128 * NJ:OS_B1F + (d + 1) * 128 * NJ]
                              .rearrange("(p a) -> p a", a=NJ))
            b1f_s.append(t1)
            t2 = pc.tile([128, NI], F32, name=f"b2f_s{d}")
            nc.sync.dma_start(t2, sml[OS_B2F + d * 128 * NI:OS_B2F + (d + 1) * 128 * NI]
                              .rearrange("(p a) -> p a", a=NI))
            b2f_s.append(t2)
            t4 = pc.tile([1, H], F32, name=f"b2r_s{d}")
            nc.sync.dma_start(t4, sml[OS_B2R + d * H:OS_B2R + (d + 1) * H]
                              .rearrange("(a b) -> a b", a=1))
            b2r_s.append(t4)

        # long-lived P1 pools claimed first so the transients below stay
        # adjacent in the ring and merge into one reusable gap
        p_k = tc.alloc_tile_pool(name="pk", bufs=1)
        p_v = tc.alloc_tile_pool(name="pv", bufs=1, side="right")

        p_wst = tc.alloc_tile_pool(name="pwst", bufs=2, side="right")
        for d in range(2):
            # mw1[d] [HE, H] fp8 -> dequant -> w1tb_d[d] = mw1[d].T [H, HE] bf16
            for hi in range(NI):
                src = p_wst.tile([128, NJ, 128], F8, name=f"w1s{d}{hi}", tag="w1src")
                nc.sync.dma_start(src, wv_w1(d)[:, hi * 128:(hi + 1) * 128]
                                  .rearrange("(j p) c -> p j c", p=128))
                strip = p_wst.tile([128, HE], BF16, name=f"w1str{d}{hi}", tag="w1str")
                for j in range(NJ):
                    deq = p_wst.tile([128, 128], BF16, name=f"wdq{d}{hi}{j}", tag="wdq")
                    nc.scalar.activation(deq, src[:, j, :], AF.Copy)
                    pt = pstr(BF16)
                    nc.tensor.transpose(pt, deq, ident_b)
                    nc.scalar.activation(strip[:, j * 128:(j + 1) * 128], pt, AF.Copy)
                nc.sync.dma_start(w1tb_d[d][hi * 128:(hi + 1) * 128, :], strip)
        for d in range(2):
            # mw2[d] [H, HE] fp8 -> dequant -> w2tb_d[d] = mw2[d].T [HE, H] bf16
            for jt in range(NJ):
                src = p_wst.tile([128, NI, 128], F8, name=f"w2s{d}{jt}", tag="w2src")
                nc.sync.dma_start(src, wv_w2(d)[:, jt * 128:(jt + 1) * 128]
                                  .rearrange("(i p) c -> p i c", p=128))
                strip = p_wst.tile([128, H], BF16, name=f"w2str{d}{jt}", tag="w2str")
                for i in range(NI):
                    deq = p_wst.tile([128, 128], BF16, name=f"wdq2{d}{jt}{i}", tag="wdq")
                    nc.scalar.activation(deq, src[:, i, :], AF.Copy)
                    pt = pstr(BF16)
                    nc.tensor.transpose(pt, deq, ident_b)
                    nc.scalar.activation(strip[:, i * 128:(i + 1) * 128], pt, AF.Copy)
                nc.sync.dma_start(w2tb_d[d][jt * 128:(jt + 1) * 128, :], strip)
        # natural-layout bf16 copies for the backward passes
        for jt in range(NJ):
            t8 = p_wst.tile([128, H], F8, name=f"n1i{jt}", tag="n1i")
            nc.sync.dma_start(t8, wv_w1(1)[jt * 128:(jt + 1) * 128, :])
            tb = p_wst.tile([128, H], BF16, name=f"n1b{jt}", tag="n1b")
            nc.scalar.activation(tb, t8, AF.Copy)
            nc.sync.dma_start(w1n1_d[jt * 128:(jt + 1) * 128, :], tb)
        for d in range(2):
            for ot in range(NI):
                t8 = p_wst.tile([128, HE], F8, name=f"n2i{d}{ot}", tag="n2i")
                nc.sync.dma_start(t8, wv_w2(d)[ot * 128:(ot + 1) * 128, :])
                tb = p_wst.tile([128, HE], BF16, name=f"n2b{d}{ot}", tag="n2b")
                nc.scalar.activation(tb, t8, AF.Copy)
                nc.sync.dma_start(w2nb_d[d][ot * 128:(ot + 1) * 128, :], tb)
        p_wst.release()

        # =======================================================
        # P1: projections q/k/v + gates   (bf16 x, bf16 weights)
        # =======================================================
        k_fb = [p_k.tile([128, T], BF16, name=f"k_fb{i}") for i in range(NI)]
        k_tb = [p_k.tile([128, H], BF16, name=f"k_tb{t}") for t in range(NT)]

        p_x = tc.alloc_tile_pool(name="px", bufs=1)
        x_f = [p_x.tile([128, T], BF16, name=f"x_f{it}") for it in range(NI)]
        p_xt = tc.alloc_tile_pool(name="pxt", bufs=2, side="right")
        for tb in range(NT):
            xq = p_xt.tile([128, H], I8, name=f"xq{tb}", tag="xq")
            nc.sync.dma_start(xq, xqv[tb * 128:(tb + 1) * 128, :])
            xs = p_xt.tile([128, 1], F32, name=f"xs{tb}", tag="xs")
            nc.sync.dma_start(xs, xsv[tb * 128:(tb + 1) * 128, :])
            xt = p_xt.tile([128, H], BF16, name=f"xt{tb}", tag="xt")
            nc.scalar.activation(xt, xq, AF.Copy, scale=xs)
            for it in range(NI):
                pt = pstr(BF16)
                nc.tensor.transpose(pt, xt[:, it * 128:(it + 1) * 128], ident_b)
                nc.scalar.activation(x_f[it][:, tb * 128:(tb + 1) * 128], pt, AF.Copy)
        p_xt.release()

        p_wp = tc.alloc_tile_pool(name="pwp", bufs=1)
        wq_s = []
        wk_s = []
        wv_s = []
        for it in range(NI):
            t = p_wp.tile([128, H], BF16, name=f"wq_s{it}")
            nc.sync.dma_start(t, wv_pq[it * 128:(it + 1) * 128, :])
            wq_s.append(t)
            t = p_wp.tile([128, H], BF16, name=f"wk_s{it}")
            nc.sync.dma_start(t, wv_pk[it * 128:(it + 1) * 128, :])
            wk_s.append(t)
            t = p_wp.tile([128, H], BF16, name=f"wv_s{it}")
            nc.sync.dma_start(t, wv_pv[it * 128:(it + 1) * 128, :])
            wv_s.append(t)
        bq_s = p_wp.tile([1, H], BF16, name="bq_s")
        nc.sync.dma_start(bq_s, wall[OFBQ:OFBQ + H].rearrange("(a b) -> a b", a=1))
        bk_s = p_wp.tile([1, H], BF16, name="bk_s")
        nc.sync.dma_start(bk_s, wall[OFBK:OFBK + H].rearrange("(a b) -> a b", a=1))
        vb_s = p_wp.tile([1, H], BF16, name="vb_s")
        nc.sync.dma_start(vb_s, wall[OFVB:OFVB + H].rearrange("(a b) -> a b", a=1))

        v_t = [p_v.tile([128, H], F32, name=f"v_t{t}") for t in range(NT)]

        gsum_p = psax("gsum_p")

        for tb in range(NT):
            ts = slice(tb * 128, (tb + 1) * 128)
            # ---- gates ----
            pg = psmm()
            mm_group(pg[:, 0:4], [(x_f[it][:, ts], gw_s[:, 4 * it:4 * it + 4]) for it in range(NI)],
                     bias=(ones_r_b, gb_s))
            sig = p_scr.tile([128, 3], F32, name=f"sig{tb}", tag="sig")
            nc.scalar.activation(sig, pg[:, 0:3], AF.Sigmoid)
            nc.vector.tensor_scalar_mul(m_t[tb], sig[:, 0:1], CC)
            nc.tensor.matmul(gsum_p[0:1, 0:3], ones_c_f, sig,
                             start=(tb == 0), stop=(tb == NT - 1))

            # ---- q ----
            pq = psmm()
            mm_group(pq, [(x_f[it][:, ts], wq_s[it]) for it in range(NI)],
                     bias=(ones_r_b, bq_s))
            sqq = p_scr.tile([128, 1], F32, name="sqq", tag="sq1")
            scq = p_scr.tile([128, 512], F32, name="scq", tag="s512")
            nc.scalar.activation(scq, pq, AF.Square, accum_out=sqq)
            nrq = p_scr.tile([128, 1], F32, name="nrq", tag="nr1")
            nc.scalar.activation(nrq, sqq, AF.Sqrt)
            nc.vector.tensor_scalar_max(nrq, nrq, 1e-12)
            rnq = p_scr.tile([128, 1], F32, name="rnq", tag="rn1")
            nc.vector.reciprocal(rnq, nrq)
            qt_tile = p_scr.tile([128, 512], F32, name="qt_tile", tag="qt")
            nc.vector.tensor_scalar_mul(qt_tile, pq, rnq)
            nc.scalar.dma_start(qt_d[ts, :], qt_tile)
            for it in range(NI):
                ptq = pstr()
                nc.tensor.transpose(ptq, qt_tile[:, it * 128:(it + 1) * 128], ident_f)
                qfs = p_scr.tile([128, 128], F32R, name="qfs", tag="qfs")
                nc.scalar.activation(qfs, ptq, AF.Copy)
                nc.scalar.dma_start(qf_d[it * 128:(it + 1) * 128, ts], qfs)

            # ---- k ----
            pk = psmm()
            mm_group(pk, [(x_f[it][:, ts], wk_s[it]) for it in range(NI)],
                     bias=(ones_r_b, bk_s))
            sqk = p_scr.tile([128, 1], F32, name="sqk", tag="sq1")
            sck = p_scr.tile([128, 512], F32, name="sck", tag="s512")
            nc.scalar.activation(sck, pk, AF.Square, accum_out=sqk)
            nrk = p_scr.tile([128, 1], F32, name="nrk", tag="nr1")
            nc.scalar.activation(nrk, sqk, AF.Sqrt)
            nc.vector.tensor_scalar_max(nrk, nrk, 1e-12)
            rnk = p_scr.tile([128, 1], F32, name="rnk", tag="rn1")
            nc.vector.reciprocal(rnk, nrk)
            nc.vector.tensor_scalar_mul(k_tb[tb], pk, rnk)
            for it in range(NI):
                ptk = pstr(BF16)
                nc.tensor.transpose(ptk, k_tb[tb][:, it * 128:(it + 1) * 128], ident_b)
                nc.scalar.activation(k_fb[it][:, ts], ptk, AF.Copy)

            # ---- v ----
            pv = psmm()
            mm_group(pv, [(x_f[it][:, ts], wv_s[it]) for it in range(NI)],
                     bias=(ones_r_b, vb_s))
            nc.vector.tensor_copy(v_t[tb], pv)

        gsum_s = pc.tile([1, 3], F32, name="gsum_s")
        nc.scalar.activation(gsum_s, gsum_p[0:1, 0:3], AF.Copy)
        nc.gpsimd.dma_start(ar0_in, gsum_s)
        nc.gpsimd.collective_compute(
            "AllReduce", ADD, replica_groups=[list(range(NCORE))],
            ins=[ar0_in.opt()], outs=[ar0_out.opt()])

        p_wp.release()
        p_x.release()

        # =======================================================
        # P2: forward k-path layer 0 (bf16)
        # =======================================================
        p_w1tb0 = tc.alloc_tile_pool(name="pw1tb0", bufs=1)
        w1tb0 = []
        for it in range(NI):
            t = p_w1tb0.tile([128, HE], BF16, name=f"w1tb0{it}")
            (nc.sync if it % 2 == 0 else nc.gpsimd).dma_start(t, w1tb_d[0][it * 128:(it + 1) * 128, :])
            w1tb0.append(t)
        p_w1tb1 = tc.alloc_tile_pool(name="pw1tb1", bufs=1)
        w1tb1 = []
        for it in range(NI):
            t = p_w1tb1.tile([128, HE], BF16, name=f"w1tb1{it}")
            (nc.gpsimd if it % 2 == 0 else nc.sync).dma_start(t, w1tb_d[1][it * 128:(it + 1) * 128, :])
            w1tb1.append(t)
        p_x1 = tc.alloc_tile_pool(name="px1", bufs=1)
        x1f = [p_x1.tile([128, T], BF16, name=f"x1f{i}") for i in range(NI)]
        x1t = [p_x1.tile([128, H], BF16, name=f"x1t{t}") for t in range(NT)]
        p_w2tb1 = tc.alloc_tile_pool(name="pw2tb1", bufs=1)
        w2tb1 = []
        for jt in range(NJ):
            t = p_w2tb1.tile([128, H], BF16, name=f"w2tb1{jt}")
            (nc.sync if jt % 2 == 0 else nc.gpsimd).dma_start(t, w2tb_d[1][jt * 128:(jt + 1) * 128, :])
            w2tb1.append(t)
        p_w2tb0 = tc.alloc_tile_pool(name="pw2tb0", bufs=1)
        w2tb0 = []
        for jt in range(NJ):
            t = p_w2tb0.tile([128, H], BF16, name=f"w2tb0{jt}")
            (nc.gpsimd if jt % 2 == 0 else nc.sync).dma_start(t, w2tb_d[0][jt * 128:(jt + 1) * 128, :])
            w2tb0.append(t)

        p_h0 = tc.alloc_tile_pool(name="ph0", bufs=1)
        h0f = [p_h0.tile([128, T], BF16, name=f"h0f{j}") for j in range(NJ)]
        for jt in range(NJ):
            for th in range(TH):
                hs = slice(th * 512, (th + 1) * 512)
                ph = psmm()
                mm_group(ph, [(w1tb0[it][:, jt * 128:(jt + 1) * 128], k_fb[it][:, hs])
                              for it in range(NI)])
                nc.scalar.activation(h0f[jt][:, hs], ph, AF.Silu,
                                     bias=b1f_s[0][:, jt:jt + 1])

        for it in range(NI):
            for th in range(TH):
                hs = slice(th * 512, (th + 1) * 512)
                px = psmm()
                mm_group(px, [(w2tb0[jt][:, it * 128:(it + 1) * 128], h0f[jt][:, hs])
                              for jt in range(NJ)])
                nc.vector.scalar_tensor_tensor(x1f[it][:, hs], px, b2f_s[0][:, it:it + 1],
                                               k_fb[it][:, hs], ADD, ADD)
        for tb in range(NT):
            ts = slice(tb * 128, (tb + 1) * 128)
            px = psmm()
            mm_group(px, [(h0f[jt][:, ts], w2tb0[jt]) for jt in range(NJ)],
                     bias=(ones_r_b, b2rb_s))
            nc.vector.tensor_tensor(x1t[tb], px, k_tb[tb], ADD)

        p_h0.release()
        p_w2tb0.release()

        # =======================================================
        # P3: forward layer 1 + g2
        # =======================================================
        p_h1 = tc.alloc_tile_pool(name="ph1", bufs=1)
        h1f = [p_h1.tile([128, T], BF16, name=f"h1f{j}") for j in range(NJ)]
        for jt in range(NJ):
            for th in range(TH):
                hs = slice(th * 512, (th + 1) * 512)
                ph = psmm()
                mm_group(ph, [(w1tb1[it][:, jt * 128:(jt + 1) * 128], x1f[it][:, hs])
                              for it in range(NI)])
                nc.scalar.activation(h1f[jt][:, hs], ph, AF.Silu,
                                     bias=b1f_s[1][:, jt:jt + 1])

        p_g2 = tc.alloc_tile_pool(name="pg2", bufs=1, side="right")
        g2t = [p_g2.tile([128, H], BF16, name=f"g2t{t}") for t in range(NT)]
        g2f = [p_g2.tile([128, T], BF16, name=f"g2f{i}") for i in range(NI)]
        db21_p = psax("db21_p")
        for tb in range(NT):
            ts = slice(tb * 128, (tb + 1) * 128)
            px = psmm()
            mm_group(px, [(h1f[jt][:, ts], w2tb1[jt]) for jt in range(NJ)])
            sc1 = p_scr.tile([128, 512], F32, name="sc1", tag="s512")
            nc.vector.tensor_sub(sc1, px, v_t[tb])
            nc.vector.tensor_tensor(sc1, sc1, x1t[tb], ADD)
            nc.vector.tensor_scalar_mul(g2t[tb], sc1, m_t[tb])
            nc.tensor.matmul(db21_p[0:1, 0:512], ones_c_b, g2t[tb],
                             start=(tb == 0), stop=(tb == NT - 1))
            for ot in range(NI):
                ptg = pstr(BF16)
                nc.tensor.transpose(ptg, g2t[tb][:, ot * 128:(ot + 1) * 128], ident_b)
                nc.scalar.activation(g2f[ot][:, ts], ptg, AF.Copy)

        nc.scalar.activation(db21r, db21_p[0:1, 0:512], AF.Copy)
        nc.sync.dma_start(arview_b2(ar1_in), db21r)

        p_h1.release()
        p_w2tb1.release()

        # =======================================================
        # P4: backward layer 1 (4 chunks over HE)
        # =======================================================
        p_gx1 = tc.alloc_tile_pool(name="pgx1", bufs=1, side="right")
        gx1f = [p_gx1.tile([128, T], F32, name=f"gx1f{i}") for i in range(NI)]
        for it in range(NI):
            nc.scalar.activation(gx1f[it], g2f[it], AF.Copy)

        p_ch = tc.alloc_tile_pool(name="pch", bufs=1, side="right")
        h1c = [p_ch.tile([128, CW], BF16, name=f"h1c{t}") for t in range(NT)]
        gp1c = [p_ch.tile([128, CW], BF16, name=f"gp1c{t}") for t in range(NT)]
        gp1f = [p_ch.tile([128, T], BF16, name=f"gp1f{j}") for j in range(NCH)]

        p_nat1a = tc.alloc_tile_pool(name="pnat1a", bufs=1)
        w1n1b = []
        for jt in range(NJ):
            t = p_nat1a.tile([128, H], BF16, name=f"w1n1b{jt}")
            (nc.sync if jt % 2 == 0 else nc.gpsimd).dma_start(t, w1n1_d[jt * 128:(jt + 1) * 128, :])
            w1n1b.append(t)
        p_nat1b = tc.alloc_tile_pool(name="pnat1b", bufs=1)
        w2n1b = []
        for ot in range(NI):
            t = p_nat1b.tile([128, HE], BF16, name=f"w2n1b{ot}")
            (nc.gpsimd if ot % 2 == 0 else nc.sync).dma_start(t, w2nb_d[1][ot * 128:(ot + 1) * 128, :])
            w2n1b.append(t)

        for c in range(NCH):
            cs = slice(c * CW, (c + 1) * CW)
            for tb in range(NT):
                ts = slice(tb * 128, (tb + 1) * 128)
                p1 = psmm()
                mm_group(p1, [(x1f[it][:, ts], w1tb1[it][:, cs]) for it in range(NI)],
                         bias=(ones_r_b, b1rb_s[1][:, cs]))
                nc.scalar.activation(h1c[tb], p1, AF.Silu)
                nc.scalar.activation(gp1c[tb], p1, AF.Derivative_silu)
                p2 = psmm()
                mm_group(p2, [(g2f[ot][:, ts], w2n1b[ot][:, cs]) for ot in range(NI)])
                nc.vector.tensor_tensor(gp1c[tb], p2, gp1c[tb], MULT)

            # dW2T_1 rows of this chunk
            for js in range(4):
                pw = psmm()
                mm_group(pw, [(h1c[tb][:, js * 128:(js + 1) * 128], g2t[tb])
                              for tb in range(NT)])
                wst = p_scr.tile([128, 512], BF16, name="wst", tag="wst")
                nc.scalar.activation(wst, pw, AF.Copy)
                nc.sync.dma_start(
                    arview_w2(ar1_in)[(c * 4 + js) * 128:(c * 4 + js + 1) * 128, :], wst)
            # dW1T_1 columns of this chunk
            for ib in range(NI):
                pw = psmm()
                mm_group(pw, [(x1t[tb][:, ib * 128:(ib + 1) * 128], gp1c[tb])
                              for tb in range(NT)])
                wst = p_scr.tile([128, 512], BF16, name="wst2", tag="wst")
                nc.scalar.activation(wst, pw, AF.Copy)
                nc.sync.dma_start(
                    arview_w1(ar1_in)[ib * 128:(ib + 1) * 128, cs], wst)
            # db1_1 chunk
            pb = psax(f"db11_p{c}")
            mm_group(pb[0:1, 0:CW], [(ones_c_b, gp1c[tb]) for tb in range(NT)])
            dbr = p_scr.tile([1, CW], BF16, name=f"db11r{c}", tag="dbr")
            nc.scalar.activation(dbr, pb[0:1, 0:CW], AF.Copy)
            nc.sync.dma_start(arview_b1(ar1_in)[:, cs], dbr)
            # gpre1 transposed (F layout) for gx1 chain
            for tb in range(NT):
                ts = slice(tb * 128, (tb + 1) * 128)
                for js in range(4):
                    ptp = pstr(BF16)
                    nc.tensor.transpose(ptp, gp1c[tb][:, js * 128:(js + 1) * 128], ident_b)
                    nc.scalar.activation(gp1f[js][:, ts], ptp, AF.Copy)
            # gx1 += gpre1 @ W1n[1]
            for ib in range(NI):
                for th in range(TH):
                    hs = slice(th * 512, (th + 1) * 512)
                    pg = psmm()
                    mm_group(pg, [(w1n1b[c * 4 + js][:, ib * 128:(ib + 1) * 128],
                                   gp1f[js][:, hs]) for js in range(4)])
                    nc.vector.tensor_tensor(gx1f[ib][:, hs], gx1f[ib][:, hs], pg, ADD)

        nc.gpsimd.collective_compute(
            "AllReduce", ADD, replica_groups=[list(range(NCORE))],
            ins=[ar1_in.opt()], outs=[ar1_out.opt()])

        p_nat1b.release()
        p_nat1a.release()
        p_x1.release()
        p_w1tb1.release()

        # =======================================================
        # P5: backward layer 0
        # =======================================================
        p_w2n0b = tc.alloc_tile_pool(name="pw2n0b", bufs=1)
        w2n0b = []
        for ot in range(NI):
            t = p_w2n0b.tile([128, HE], BF16, name=f"w2n0b{ot}")
            (nc.sync if ot % 2 == 0 else nc.gpsimd).dma_start(t, w2nb_d[0][ot * 128:(ot + 1) * 128, :])
            w2n0b.append(t)

        p_gx1b = tc.alloc_tile_pool(name="pgx1b", bufs=1, side="right")
        gx1fb = [p_gx1b.tile([128, T], BF16, name=f"gx1fb{i}") for i in range(NI)]
        gx1t = [p_gx1b.tile([128, H], BF16, name=f"gx1t{t}") for t in range(NT)]
        for it in range(NI):
            nc.scalar.activation(gx1fb[it], gx1f[it], AF.Copy)
        for tb in range(NT):
            ts = slice(tb * 128, (tb + 1) * 128)
            for ib in range(NI):
                ptx = pstr()
                nc.tensor.transpose(ptx, gx1f[ib][:, ts], ident_f)
                nc.vector.tensor_copy(gx1t[tb][:, ib * 128:(ib + 1) * 128], ptx)

        db20_p = psax("db20_p")
        mm_group(db20_p[0:1, 0:512], [(ones_c_b, gx1t[tb]) for tb in range(NT)])
        nc.scalar.activation(db20r, db20_p[0:1, 0:512], AF.Copy)
        nc.sync.dma_start(arview_b2(ar2_in), db20r)

        h0c = [p_ch.tile([128, CW], BF16, name=f"h0c{t}", tag=f"h1c{t}") for t in range(NT)]
        gp0c = [p_ch.tile([128, CW], BF16, name=f"gp0c{t}", tag=f"gp1c{t}") for t in range(NT)]

        for c in range(NCH):
            cs = slice(c * CW, (c + 1) * CW)
            for tb in range(NT):
                ts = slice(tb * 128, (tb + 1) * 128)
                p1 = psmm()
                mm_group(p1, [(k_fb[it][:, ts], w1tb0[it][:, cs]) for it in range(NI)],
                         bias=(ones_r_b, b1rb_s[0][:, cs]))
                nc.scalar.activation(h0c[tb], p1, AF.Silu)
                nc.scalar.activation(gp0c[tb], p1, AF.Derivative_silu)
                p2 = psmm()
                mm_group(p2, [(gx1fb[ot][:, ts], w2n0b[ot][:, cs]) for ot in range(NI)])
                nc.vector.tensor_tensor(gp0c[tb], p2, gp0c[tb], MULT)
            for js in range(4):
                pw = psmm()
                mm_group(pw, [(h0c[tb][:, js * 128:(js + 1) * 128], gx1t[tb])
                              for tb in range(NT)])
                wst = p_scr.tile([128, 512], BF16, name="wst3", tag="wst")
                nc.scalar.activation(wst, pw, AF.Copy)
                nc.sync.dma_start(
                    arview_w2(ar2_in)[(c * 4 + js) * 128:(c * 4 + js + 1) * 128, :], wst)
            for ib in range(NI):
                pw = psmm()
                mm_group(pw, [(k_tb[tb][:, ib * 128:(ib + 1) * 128], gp0c[tb])
                              for tb in range(NT)])
                wst = p_scr.tile([128, 512], BF16, name="wst4", tag="wst")
                nc.scalar.activation(wst, pw, AF.Copy)
                nc.sync.dma_start(
                    arview_w1(ar2_in)[ib * 128:(ib + 1) * 128, cs], wst)
            pb = psax(f"db10_p{c}")
            mm_group(pb[0:1, 0:CW], [(ones_c_b, gp0c[tb]) for tb in range(NT)])
            dbr = p_scr.tile([1, CW], BF16, name=f"db10r{c}", tag="dbr")
            nc.scalar.activation(dbr, pb[0:1, 0:CW], AF.Copy)
            nc.sync.dma_start(arview_b1(ar2_in)[:, cs], dbr)

        nc.gpsimd.collective_compute(
            "AllReduce", ADD, replica_groups=[list(range(NCORE))],
            ins=[ar2_in.opt()], outs=[ar2_out.opt()])

        p_w2n0b.release()
        p_w1tb0.release()
        p_k.release()
        p_gx1b.release()
        p_ch.release()
        p_gx1.release()
        p_g2.release()
        p_v.release()

        # =======================================================
        # P6/P7: fused weight update + final forward on q
        # stage A: depth 0, stage B: depth 1
        # =======================================================
        gs = pc.tile([1, 3], F32, name="gs")
        nc.gpsimd.dma_start(gs, ar0_out)
        s_sc = pc.tile([1, 1], F32, name="s_sc")
        nc.vector.tensor_scalar(s_sc, gs[:, 1:2], -1.0 / BS, 1.0, MULT, ADD)
        tb_sc = pc.tile([1, 1], F32, name="tb_sc")
        nc.vector.tensor_scalar_mul(tb_sc, gs[:, 0:1], 0.1 / BS)
        pb1 = psax("pb1")
        nc.tensor.matmul(pb1[:, 0:1], ones_r_f, s_sc, start=True, stop=True)
        nc.tensor.matmul(pb1[:, 1:2], ones_r_f, tb_sc, start=True, stop=True)
        s_bc = pc.tile([128, 1], F32, name="s_bc")
        nc.scalar.activation(s_bc, pb1[:, 0:1], AF.Copy)
        tb_bc = pc.tile([128, 1], F32, name="tb_bc")
        nc.scalar.activation(tb_bc, pb1[:, 1:2], AF.Copy)

        # ---- stage A (depth 0; grads in ar2_out) ----
        p_x1q = tc.alloc_tile_pool(name="px1q", bufs=1)
        x1qf = [p_x1q.tile([128, T], F32R, name=f"x1qf{i}") for i in range(NI)]
        x1qt = [p_x1q.tile([128, H], F32, name=f"x1qt{t}") for t in range(NT)]

        p_w0 = tc.alloc_tile_pool(name="pw0", bufs=1)
        p_rot = tc.alloc_tile_pool(name="prot", bufs=2)
        w10 = []
        for it in range(NI):
            t = p_w0.tile([128, HE], F32R, name=f"w10_{it}")
            for cb in range(NCH):
                cs = slice(cb * CW, (cb + 1) * CW)
                rb = p_rot.tile([128, CW], BF16, name=f"r10_{it}_{cb}", tag="rot")
                (nc.sync if cb % 2 == 0 else nc.gpsimd).dma_start(rb, w1tb_d[0][it * 128:(it + 1) * 128, cs])
                nc.scalar.activation(t[:, cs], rb, AF.Copy)
            w10.append(t)
        w20 = []
        for jt in range(NJ):
            rb = p_rot.tile([128, H], BF16, name=f"r20_{jt}", tag="rot")
            (nc.gpsimd if jt % 2 == 0 else nc.sync).dma_start(rb, w2tb_d[0][jt * 128:(jt + 1) * 128, :])
            t = p_w0.tile([128, H], F32R, name=f"w20_{jt}")
            nc.scalar.activation(t, rb, AF.Copy)
            w20.append(t)

        def update_weights(w1x, w2x, arw, d, pu):
            for it in range(NI):
                for cb in range(NCH):
                    cs = slice(cb * CW, (cb + 1) * CW)
                    g1 = pu.tile([128, CW], BF16, name=f"g1_{d}_{it}_{cb}", tag="g1")
                    nc.sync.dma_start(g1, arview_w1(arw)[it * 128:(it + 1) * 128, cs])
                    t1 = pu.tile([128, CW], F32, name=f"t1_{d}_{it}_{cb}", tag="t1")
                    nc.scalar.activation(t1, g1, AF.Copy, scale=tb_bc)
                    nc.vector.scalar_tensor_tensor(w1x[it][:, cs], w1x[it][:, cs],
                                                   s_bc, t1, MULT, SUB)
            for jt in range(NJ):
                g2_ = pu.tile([128, H], BF16, name=f"g2_{d}_{jt}", tag="g2")
                nc.sync.dma_start(g2_, arview_w2(arw)[jt * 128:(jt + 1) * 128, :])
                t2 = pu.tile([128, H], F32, name=f"t2_{d}_{jt}", tag="t2")
                nc.scalar.activation(t2, g2_, AF.Copy, scale=tb_bc)
                nc.vector.scalar_tensor_tensor(w2x[jt], w2x[jt], s_bc, t2, MULT, SUB)
            gb1 = pu.tile([128, NJ], BF16, name=f"gb1_{d}", tag="gb1")
            nc.sync.dma_start(gb1, arw[OF_B1:OF_B1 + HE].rearrange("(a p) -> p a", p=128))
            tb1 = pu.tile([128, NJ], F32, name=f"tb1_{d}", tag="tb1")
            nc.scalar.activation(tb1, gb1, AF.Copy, scale=tb_bc)
            nc.vector.scalar_tensor_tensor(b1f_s[d], b1f_s[d], s_bc, tb1, MULT, SUB)
            gb2 = pu.tile([128, NI], BF16, name=f"gb2_{d}", tag="gb2")
            nc.sync.dma_start(gb2, arw[OF_B2:OF_B2 + H].rearrange("(a p) -> p a", p=128))
            tb2 = pu.tile([128, NI], F32, name=f"tb2_{d}", tag="tb2")
            nc.scalar.activation(tb2, gb2, AF.Copy, scale=tb_bc)
            nc.vector.scalar_tensor_tensor(b2f_s[d], b2f_s[d], s_bc, tb2, MULT, SUB)
            gb2r = pu.tile([1, H], BF16, name=f"gb2r_{d}", tag="gb2r")
            nc.sync.dma_start(gb2r, arview_b2(arw))
            tb2r = pu.tile([1, H], F32, name=f"tb2r_{d}", tag="tb2r")
            nc.scalar.activation(tb2r, gb2r, AF.Copy, scale=tb_sc)
            nc.vector.scalar_tensor_tensor(b2r_s[d], b2r_s[d], s_sc, tb2r, MULT, SUB)

        p_updA = tc.alloc_tile_pool(name="pupdA", bufs=1)
        update_weights(w10, w20, ar2_out, 0, p_updA)

        p_q = tc.alloc_tile_pool(name="pq", bufs=1)
        qfh = []
        for it in range(NI):
            t = p_q.tile([128, T], F32R, name=f"qfh{it}")
            (nc.scalar if it % 2 == 0 else nc.gpsimd).dma_start(t, qf_d[it * 128:(it + 1) * 128, :])
            qfh.append(t)

        p_hq = tc.alloc_tile_pool(name="phq", bufs=1)
        p_hq2 = tc.alloc_tile_pool(name="phq2", bufs=1)
        for hb in range(TH):
            hs = slice(hb * 512, (hb + 1) * 512)
            h0q = []
            for jt in range(NJ):
                ph = psmm()
                mm_group(ph, [(w10[it][:, jt * 128:(jt + 1) * 128], qfh[it][:, hs])
                              for it in range(NI)])
                hqt = (p_hq if jt < 8 else p_hq2).tile(
                    [128, 512], F32R, name=f"h0q{jt}_{hb}", tag=f"h0q{jt}")
                nc.scalar.activation(hqt, ph, AF.Silu, bias=b1f_s[0][:, jt:jt + 1])
                h0q.append(hqt)
            for it in range(NI):
                px = psmm()
                mm_group(px, [(w20[jt][:, it * 128:(it + 1) * 128], h0q[jt])
                              for jt in range(NJ)])
                nc.vector.scalar_tensor_tensor(x1qf[it][:, hs], px, b2f_s[0][:, it:it + 1],
                                               qfh[it][:, hs], ADD, ADD)
            for tb4 in range(4):
                tbg = hb * 4 + tb4
                px = psmm()
                mm_group(px, [(h0q[jt][:, tb4 * 128:(tb4 + 1) * 128], w20[jt])
                              for jt in range(NJ)],
                         bias=(ones_r_f, b2r_s[0]))
                qtt = p_scr.tile([128, 512], F32, name=f"qtt{tbg}", tag="s512")
                nc.sync.dma_start(qtt, qt_d[tbg * 128:(tbg + 1) * 128, :])
                nc.vector.tensor_tensor(x1qt[tbg], px, qtt, ADD)

        p_hq2.release()
        p_hq.release()
        p_q.release()
        p_updA.release()
        p_rot.release()
        p_w0.release()

        # ---- stage B (depth 1; grads in ar1_out) ----
        p_w1x = tc.alloc_tile_pool(name="pw1x", bufs=1)
        p_rotb = tc.alloc_tile_pool(name="protb", bufs=2)
        w11 = []
        for it in range(NI):
            t = p_w1x.tile([128, HE], F32R, name=f"w11_{it}")
            for cb in range(NCH):
                cs = slice(cb * CW, (cb + 1) * CW)
                rb = p_rotb.tile([128, CW], BF16, name=f"r11_{it}_{cb}", tag="rot")
                (nc.sync if cb % 2 == 0 else nc.gpsimd).dma_start(rb, w1tb_d[1][it * 128:(it + 1) * 128, cs])
                nc.scalar.activation(t[:, cs], rb, AF.Copy)
            w11.append(t)
        w21 = []
        for jt in range(NJ):
            rb = p_rotb.tile([128, H], BF16, name=f"r21_{jt}", tag="rot")
            (nc.gpsimd if jt % 2 == 0 else nc.sync).dma_start(rb, w2tb_d[1][jt * 128:(jt + 1) * 128, :])
            t = p_w1x.tile([128, H], F32R, name=f"w21_{jt}")
            nc.scalar.activation(t, rb, AF.Copy)
            w21.append(t)

        p_updB = tc.alloc_tile_pool(name="pupdB", bufs=1)
        update_weights(w11, w21, ar1_out, 1, p_updB)

        p_h1q = tc.alloc_tile_pool(name="ph1q", bufs=1)
        p_h1q2 = tc.alloc_tile_pool(name="ph1q2", bufs=1)
        for hb in range(TH):
            hs = slice(hb * 512, (hb + 1) * 512)
            h1q = []
            for jt in range(NJ):
                ph = psmm()
                mm_group(ph, [(w11[it][:, jt * 128:(jt + 1) * 128], x1qf[it][:, hs])
                              for it in range(NI)])
                hqt = (p_h1q if jt < 8 else p_h1q2).tile(
                    [128, 512], F32R, name=f"h1q{jt}_{hb}", tag=f"h1q{jt}")
                nc.scalar.activation(hqt, ph, AF.Silu, bias=b1f_s[1][:, jt:jt + 1])
                h1q.append(hqt)
            for tb4 in range(4):
                tbg = hb * 4 + tb4
                py = psmm()
                mm_group(py, [(h1q[jt][:, tb4 * 128:(tb4 + 1) * 128], w21[jt])
                              for jt in range(NJ)],
                         bias=(ones_r_f, b2r_s[1]))
                nc.vector.tensor_tensor(x1qt[tbg], x1qt[tbg], py, ADD)
                # int8 per-token output: y = round(x * 127/rowmax), scale packed
                rmx = p_scr.tile([128, 1], F32, name=f"rmx{tbg}", tag="sq1")
                nc.vector.reduce_max(rmx, x1qt[tbg], axis=mybir.AxisListType.X,
                                     apply_absolute_value=True)
                nc.vector.tensor_scalar_max(rmx, rmx, 1e-20)
                ysc = p_scr.tile([128, 1], F32, name=f"ysc{tbg}", tag="nr1")
                nc.vector.tensor_scalar_mul(ysc, rmx, 1.0 / 127.0)
                yiv = p_scr.tile([128, 1], F32, name=f"yiv{tbg}", tag="rn1")
                nc.vector.reciprocal(yiv, ysc)
                yb = p_h1q2.tile([128, HW2], I8, name=f"yb{tbg}", tag="ybq")
                nc.scalar.activation(yb[:, 0:H], x1qt[tbg], AF.Copy, scale=yiv)
                nc.vector.tensor_copy(yb[:, H:H + 4].bitcast(F32), ysc)
                nc.vector.memset(yb[:, H + 4:HW2], 0)
                nc.sync.dma_start(yout[tbg * 128:(tbg + 1) * 128, :], yb)

        p_h1q2.release()
        p_h1q.release()
        p_updB.release()
        p_rotb.release()
        p_w1x.release()
        p_x1q.release()
        p_scr.release()
        pc.release()
        pp_aux.release()
        pp_tr.release()
        pp_mm.release()

    nc.finalize()
    return nc


def _get_nc():
    if "nc" not in _CACHE:
        nc = _build()
        # the module is immutable after finalize(); memoize its serialization
        # (recomputed inside every PJRT lowering otherwise, ~60ms/call)
        try:
            jb = nc.to_json_bytes()
            nc.to_json_bytes = lambda: jb
        except Exception:
            pass
        _CACHE["nc"] = nc
    return _CACHE["nc"]


_F8TBL = None


def _f8_pack(a_f32, out_i8):
    # f32 -> bf16 -> fp8 via lookup table (ml_dtypes' direct f32->fp8 cast
    # is slow); double rounding shifts values by at most one fp8 ulp
    global _F8TBL
    if _F8TBL is None:
        bits = np.arange(65536, dtype=np.uint16)
        with np.errstate(invalid="ignore"):
            _F8TBL = (bits.view(ml_dtypes.bfloat16)
                      .astype(ml_dtypes.float8_e4m3fn).view(np.uint8))
    b = np.asarray(a_f32, dtype=ml_dtypes.bfloat16).view(np.uint16).ravel()
    np.take(_F8TBL, b, out=out_i8.view(np.uint8))


def _prep(inputs):
    f32 = np.float32
    bf = ml_dtypes.bfloat16

    def g(n):
        return np.asarray(inputs[n], dtype=f32)

    x = g("x").reshape(BS, H)
    wq, bq = g("wq"), g("bq")
    wk, bk = g("wk"), g("bk")
    wv, bv = g("wv"), g("bv")
    wlr, blr = g("wlr"), g("blr")
    wf, bfg = g("wf"), g("bf")
    wm, bm = g("wm"), g("bm")
    mw1, mb1 = g("mw1"), g("mb1")
    mw2, mb2 = g("mw2"), g("mb2")

    bufs = _CACHE.setdefault("bufs", {})
    if not bufs:
        bufs["blob"] = np.zeros(WTOT, dtype=bf)
        bufs["xbuf"] = np.empty((BS, H), f32)
        bufs["xq"] = np.empty((BS, H), np.int8)
        bufs["ics"] = [np.empty(IN_N, np.int8) for _ in range(NCORE)]
    blob = bufs["blob"]
    blob8 = blob.view(np.int8)
    _f8_pack(mw1, blob8[2 * OFW1:2 * OFW1 + 2 * HEH])
    _f8_pack(mw2, blob8[2 * OFW2:2 * OFW2 + 2 * HEH])
    blob[OFPQ:OFPQ + H * H] = np.ascontiguousarray(wq.T).astype(bf).ravel()
    blob[OFPK:OFPK + H * H] = np.ascontiguousarray(wk.T).astype(bf).ravel()
    blob[OFPV:OFPV + H * H] = np.ascontiguousarray(wv.T).astype(bf).ravel()
    gwmat = np.concatenate([wlr, wf, wm, np.zeros((1, H), f32)], axis=0).T  # [H,4]
    blob[OFGW:OFGW + 4 * H] = np.ascontiguousarray(gwmat).astype(bf).ravel()
    blob[OFGB:OFGB + 4] = np.array([blr[0], bfg[0], bm[0], 0.0], f32).astype(bf)
    blob[OFBQ:OFBQ + H] = bq.astype(bf)
    blob[OFBK:OFBK + H] = bk.astype(bf)
    blob[OFVB:OFVB + H] = (bv - mb2[1]).astype(bf)
    blob[OFMB1:OFMB1 + 2 * HE] = mb1.astype(bf).ravel()
    blob[OFMB20:OFMB20 + H] = mb2[0].astype(bf)

    sm = np.zeros(SMN, dtype=f32)
    sm[OS_B1F:OS_B1F + 2 * 128 * NJ] = \
        mb1.reshape(2, NJ, 128).transpose(0, 2, 1).ravel()
    sm[OS_B2F:OS_B2F + 2 * 128 * NI] = \
        mb2.reshape(2, NI, 128).transpose(0, 2, 1).ravel()
    sm[OS_B2R:OS_B2R + 2 * H] = mb2.ravel()
    blob[OFSML:OFSML + 2 * SMN] = sm.view(bf)

    # int8 per-token x (in-place passes through persistent buffers)
    mx = np.maximum(x.max(axis=1), -x.min(axis=1))
    xsc = np.where(mx == 0, 1.0, mx * (1.0 / 127.0)).astype(f32)
    xb = bufs["xbuf"]
    np.multiply(x, (1.0 / xsc)[:, None], out=xb)
    # |x/s| <= 127*(1+2^-22), so rint stays within int8 range without a clip
    np.rint(xb, out=xb)
    xq_all = bufs["xq"]
    np.copyto(xq_all, xb, casting="unsafe")

    in_maps = []
    for cid in range(NCORE):
        ic = bufs["ics"][cid]
        ic[0:XN] = xq_all[cid * T:(cid + 1) * T].ravel()
        ic[XSO:XSO + 4 * T] = xsc[cid * T:(cid + 1) * T].view(np.int8)
        ic[WBO:WBO + 2 * WSH] = blob8[cid * 2 * WSH:(cid + 1) * 2 * WSH]
        in_maps.append({"inp": ic})
    return in_maps


def kernel(**inputs):
    nc = _get_nc()
    in_maps = _prep(inputs)
    res = run_bass_kernel_spmd(nc, in_maps, list(range(NCORE)))
    out = np.empty((BS, H), np.float32)
    for cid in range(NCORE):
        yr = np.asarray(res.results[cid]["y"])                      # [T, H+8] int8
        sc = np.ascontiguousarray(yr[:, H:H + 4]).view(np.float32)  # [T, 1]
        np.multiply(yr[:, 0:H], sc, out=out[cid * T:(cid + 1) * T])
    return out.reshape(B, S, H)
